# revision 46
# baseline (speedup 1.0000x reference)
"""Trainium2 Bass kernel for nn_GCMC (GNN message passing / GCMC scoring).

v5 strategy: row-shard users AND items across 8 NeuronCores (256 padded rows
each). Message passing is ONE merged ReduceScatter: each core column-shards
M (its 256 v-columns of M_u, u-columns of M_v), computes partial hidden sums
for ALL opposite-side rows from its local projection slice, and a single
ReduceScatter (add) over a [NC, 2, H, R, 258] fp16 payload returns both
sides' per-core hidden rows (side-branch BatchNorm partial sums ride in 2
extra columns). One slim AllGather then shares the pre-BN v-side cat output
y_v plus both sides' cat BatchNorm partial sums, so every core computes
global BatchNorm stats locally and the full embed_v for the final bilinear
score.

Precision: fp16 operands (the PE accumulates in fp32; fp16 measured no worse
than bf16 here), fp32 catT is not needed (fp16 catT/w_cat measured fine), M
and the projection copy it contracts with travel as fp8e4m3 (x64 / x4
scales, undone by the hidden-relu activation scale 2^-8) enabling DoubleRow
matmuls and halving the dominant M DMA traffic. The RS payload is fp16; the
AllGather payload MUST stay fp32 - 16-bit AllGather payloads measurably
degrade (~1.5% extra score error, consistent with the collective
round-tripping 16-bit data through bf16), while the fp16 ReduceScatter only
costs ~0.35%. Measured end-to-end max-rel error: 1.41% vs the 2% gate.

Collectives: 1x fp16 ReduceScatter (23.3us) + 1x fp32 AllGather (30.6us) on
the serial collective device, vs 2 RS + 1 AG = 77us in v2.1. A dummy Sqrt
activation at t=0 preloads the activation-function table so the BatchNorm
Sqrt does not pay a 1.3us table load on the post-RS critical path.
"""
import sys
if '/opt/trn_rl_repo' not in sys.path:
    sys.path.insert(0, '/opt/trn_rl_repo')

import numpy as np
import ml_dtypes

import concourse.bass as bass
import concourse.bacc as bacc
import concourse.mybir as mybir
import concourse.tile as tile
from concourse import bass_utils

F16N = np.float16
F8N = ml_dtypes.float8_e4m3
F32 = mybir.dt.float32
F16 = mybir.dt.float16
FP8 = mybir.dt.float8e4
AF = mybir.ActivationFunctionType
ALU = mybir.AluOpType
AXX = mybir.AxisListType.X
DR = mybir.MatmulPerfMode.DoubleRow

U = V = F = 2000
R, H, O, SH, SF = 5, 64, 75, 64, 128
RH = R * H           # 320
UP = 2048            # padded U/V/F
S = 256              # rows per core
SP2 = S + 2          # RS payload row width (256 data + 2 BN-sum cols)
NC = 8
KT = 16              # 128-row k-tiles over the padded 2048 contraction dims
EPS = 1e-5
M_SC, P_SC = 64.0, 4.0          # fp8 scales for M and prevh
HID_SC = 1.0 / (M_SC * P_SC)    # 2^-8, folded into hidden relu
NTILES = [(0, 512), (512, 512), (1024, 512), (1536, 464)]  # score v-tiles

_CACHE = {}


def _build():
    nc = bacc.Bacc("TRN2", target_bir_lowering=False, debug=False,
                   num_devices=NC)

    def din(name, shape, dt):
        return nc.dram_tensor(name, list(shape), dt, kind="ExternalInput").ap()

    fuT_d = din("fuT", (128, KT, S), F16)     # my u rows, [f, kt, u]
    fvT_d = din("fvT", (128, KT, S), F16)
    muT_d = din("muT", (R, 128, 2, UP), FP8)  # x64 M_u[r][:, my_v].T packed
    mvT_d = din("mvT", (R, 128, 2, UP), FP8)
    w_d = din("w", (128, KT, RH), F16)
    q_d = din("q", (O, R, O), F16)
    sfT_d = din("sfT", (SF, 2, S), F16)
    wside_d = din("wside", (SF, 2, SH), F16)
    wcat_d = din("wcat", (128, 6, 2, O), F16)  # rows: pre|hidden|side
    gbs_d = din("gb_side", (SH, 4), F32)
    gbc_d = din("gb_cat", (O, 4), F32)
    ident_d = din("ident", (128, 128), F16)
    mask_d = din("mask", (SH, S), F16)

    score_d = nc.dram_tensor("score", [R, S, V], F16,
                             kind="ExternalOutput").ap()

    with tile.TileContext(nc) as tc:
        with tc.tile_pool(name="const", bufs=1) as const_p, \
             tc.tile_pool(name="big", bufs=1) as big_p, \
             tc.tile_pool(name="mring", bufs=10) as m_p, \
             tc.tile_pool(name="small", bufs=1) as sm_p, \
             tc.tile_pool(name="scoresb", bufs=3) as sc_p, \
             tc.tile_pool(name="psmm", bufs=4, space="PSUM") as psmm, \
             tc.tile_pool(name="pssc", bufs=3, space="PSUM") as pssc, \
             tc.tile_pool(name="dram", bufs=1, space="DRAM") as dram_p:

            # ============ constant/small loads (SP queue) ============
            ident = const_p.tile([128, 128], F16)
            nc.sync.dma_start(ident[:], ident_d)
            eps_t = const_p.tile([128, 1], F32)
            nc.vector.memset(eps_t[:], EPS)
            sqrt_warm = const_p.tile([128, 1], F32, name="sqrt_warm")
            nc.scalar.activation(sqrt_warm[:], eps_t[:], AF.Sqrt,
                                 bias=eps_t[:])
            sfT_sb = const_p.tile([SF, 2, S], F16)
            nc.sync.dma_start(sfT_sb[:], sfT_d)
            wside_sb = const_p.tile([SF, 2, SH], F16)
            nc.sync.dma_start(wside_sb[:], wside_d)
            q_sb = const_p.tile([O, R, O], F16)
            nc.sync.dma_start(q_sb[:], q_d)
            wcat_sb = const_p.tile([128, 6, 2, O], F16)
            nc.sync.dma_start(wcat_sb[:], wcat_d)
            gbs_sb = const_p.tile([SH, 4], F32)
            nc.sync.dma_start(gbs_sb[:], gbs_d)
            gbc_sb = const_p.tile([O, 4], F32)
            nc.sync.dma_start(gbc_sb[:], gbc_d)
            mask_sb = const_p.tile([SH, S], F16)
            nc.sync.dma_start(mask_sb[:], mask_d)

            # ============ big stream (ACT queue, exact order) ============
            fvT_sb = big_p.tile([128, KT, S], F16)
            nc.scalar.dma_start(fvT_sb[:], fvT_d)
            w_sb = big_p.tile([128, KT, RH], F16)
            nc.scalar.dma_start(w_sb[:], w_d)
            fuT_sb = big_p.tile([128, KT, S], F16)
            nc.scalar.dma_start(fuT_sb[:], fuT_d)
            muT_sb = [m_p.tile([128, 2, UP], FP8, tag="mT", name=f"muT_{r}")
                      for r in range(R)]
            mvT_sb = [m_p.tile([128, 2, UP], FP8, tag="mT", name=f"mvT_{r}")
                      for r in range(R)]
            for r in range(R):
                nc.scalar.dma_start(muT_sb[r][:], muT_d[r])
            for r in range(R):
                nc.scalar.dma_start(mvT_sb[r][:], mvT_d[r])

            # ============ collective buffers ============
            replica = [list(range(NC))]
            rs_in = dram_p.tile([NC, 2, H, R, SP2], F16)
            rs_out = dram_p.tile([2, H, R, SP2], F16)
            ag_in = dram_p.tile([O, S + 4], F32)
            ag_out = dram_p.tile([NC, O, S + 4], F32, addr_space="Shared")

            # catT: [128, 6, S] f16 per side; rows pre(0:320)|hidden|side
            catT = [big_p.tile([128, 6, S], F16, name=f"catT{sd}")
                    for sd in range(2)]
            for sd in range(2):
                nc.vector.memset(catT[sd][SH:128, 5, :], 0.0)

            # partial-hidden staging, shared by both sides via WAR reuse
            stage = big_p.tile([H, NC, R, SP2], F16, name="stage")
            nc.vector.memset(stage[:, :, :, S:SP2], 0.0)

            # ============ side matmuls + BN partial sums ============
            s_loc = sm_p.tile([SH, 2, S], F32)
            junk = sm_p.tile([128, S], F32, name="junk")

            def side_branch(sd):
                ps_s = psmm.tile([SH, S], F32, tag="mm", name="ps_side")
                nc.tensor.matmul(ps_s[:], wside_sb[:, sd, :], sfT_sb[:, sd, :],
                                 start=True, stop=True)
                nc.vector.tensor_copy(s_loc[:, sd, :], ps_s[:])
                s_sums = sm_p.tile([SH, 2], F32, tag=f"s_sums{sd}",
                                   name=f"s_sums{sd}")
                nc.vector.reduce_sum(s_sums[:, 0:1], s_loc[:, sd, :], axis=AXX)
                nc.vector.tensor_mul(junk[0:SH, :], s_loc[:, sd, :],
                                     s_loc[:, sd, :])
                nc.vector.reduce_sum(s_sums[:, 1:2], junk[0:SH, :], axis=AXX)
                # replicate into every dest shard of the RS payload (row r=0)
                for c in range(NC):
                    nc.vector.tensor_copy(stage[:, c, 0, S:S + 2], s_sums[:])

            # ============ projection: pre[row, rh] = f^T W ============
            prevh = [big_p.tile([128, 2, RH], F16, name=f"prevh_{sd}")
                     for sd in range(2)]
            prevh8 = [big_p.tile([128, 2, RH], FP8, name=f"prevh8_{sd}")
                      for sd in range(2)]

            def proj_side(sd, fT):
                pre_f16 = prevh[sd]
                for ch in range(2):
                    ps_pre = psmm.tile([128, RH], F32, tag="mm", name="ps_pre")
                    for k in range(KT):
                        nc.tensor.matmul(ps_pre[:],
                                         fT[:, k, ch * 128:(ch + 1) * 128],
                                         w_sb[:, k, :],
                                         start=(k == 0), stop=(k == KT - 1))
                    nc.vector.tensor_copy(pre_f16[:, ch, :], ps_pre[:])
                    nc.scalar.activation(prevh8[sd][:, ch, :], ps_pre[:],
                                         AF.Copy, scale=P_SC)
                    for c in range(3):
                        cw = min(128, RH - c * 128)
                        ps_t = psmm.tile([128, 128], F16, tag="mm",
                                         name="ps_t")
                        nc.tensor.transpose(
                            ps_t[0:cw, :],
                            pre_f16[:, ch, c * 128:c * 128 + cw],
                            ident[:])
                        row = c * 128
                        blk, off = divmod(row, 128)
                        nc.vector.tensor_copy(
                            catT[sd][off:off + cw, blk,
                                     ch * 128:(ch + 1) * 128],
                            ps_t[0:cw, :])

            # ==== partial hidden (fp8 DoubleRow) -> f16 stage -> DMA ====
            def partial_side(sd, mT, osd):
                for r in range(R):
                    pss = [psmm.tile([H, 2, S], F32, tag="mm", name=f"ps_p{g}")
                           for g in range(4)]
                    for g in range(4):
                        for c in range(2):
                            nc.tensor.matmul(
                                pss[g][:, c, :],
                                prevh8[osd][:, :, r * H:(r + 1) * H],
                                mT[r][:, :, (2 * g + c) * S:(2 * g + c + 1) * S],
                                start=True, stop=True, perf_mode=DR)
                    for g in range(4):
                        dst = stage[:, 2 * g:2 * g + 2, r, 0:S]
                        if g % 2 == 0:
                            nc.vector.tensor_copy(dst, pss[g][:])
                        else:
                            nc.scalar.copy(dst, pss[g][:])
                    nc.sync.dma_start(
                        rs_in[:, sd, :, r, :].rearrange("c h x -> h c x"),
                        stage[:, :, r, :])

            side_branch(0)
            proj_side(1, fvT_sb)
            partial_side(0, muT_sb, 1)
            side_branch(1)
            proj_side(0, fuT_sb)
            partial_side(1, mvT_sb, 0)
            nc.gpsimd.collective_compute("ReduceScatter", ALU.add,
                                         replica_groups=replica,
                                         ins=[rs_in.opt()],
                                         outs=[rs_out.opt()])



            # ============ BN helpers (both sides batched: [P, 2]) ======
            def bn_from_sums(tg, sums, sumsq, g_col, b_col, n, P):
                def t(nm):
                    return sm_p.tile([P, 2], F32, tag=f"{nm}_{tg}",
                                     name=f"{nm}_{tg}")
                mu = t("bn_mu")
                nc.vector.tensor_scalar_mul(mu[:], sums[:], 1.0 / n)
                e2 = t("bn_e2")
                nc.vector.tensor_scalar_mul(e2[:], sumsq[:], 1.0 / n)
                var = t("bn_var")
                nc.vector.tensor_mul(var[:], mu[:], mu[:])
                nc.vector.tensor_sub(var[:], e2[:], var[:])
                std = t("bn_std")
                nc.scalar.activation(std[:], var[:], AF.Sqrt, bias=eps_t[0:P, :])
                rstd = t("bn_rstd")
                nc.vector.reciprocal(rstd[:], std[:])
                scale = t("bn_scale")
                nc.vector.tensor_mul(scale[:], g_col, rstd[:])
                shift = t("bn_shift")
                nc.vector.tensor_mul(shift[:], mu[:], scale[:])
                nc.vector.tensor_sub(shift[:], b_col, shift[:])
                return scale, shift

            # ====== hidden relu into catT rows 320:640; side BN ======
            side_tmp = sm_p.tile([SH, 2, S], F16, name="side_tmp")
            hsum = sm_p.tile([H, 2, R, SP2], F16, name="hsum")
            nc.sync.dma_start(hsum[:], rs_out.rearrange("s h r x -> h s r x"))
            t_sums = sm_p.tile([SH, 2, 2], F32, name="t_sums")
            nc.vector.tensor_copy(t_sums[:], hsum[:, :, 0, S:S + 2])
            for sd in range(2):
                for r in range(R):
                    row = RH + r * H
                    blk, off = divmod(row, 128)
                    if r % 2 == 0:
                        nc.scalar.activation(catT[sd][off:off + H, blk, :],
                                             hsum[:, sd, r, 0:S],
                                             AF.Relu, scale=HID_SC)
                    else:
                        nc.vector.tensor_scalar(
                            catT[sd][off:off + H, blk, :], hsum[:, sd, r, 0:S],
                            HID_SC, 0.0, op0=ALU.mult, op1=ALU.max)
            sc2, sh2 = bn_from_sums("sB", t_sums[:, :, 0], t_sums[:, :, 1],
                                    gbs_sb[:, 0:2], gbs_sb[:, 2:4], U, SH)
            for sd in range(2):
                nc.scalar.activation(side_tmp[:, sd, :], s_loc[:, sd, :],
                                     AF.Relu, bias=sh2[:, sd:sd + 1],
                                     scale=sc2[:, sd:sd + 1])
                nc.vector.tensor_mul(catT[sd][0:SH, 5, :], side_tmp[:, sd, :],
                                     mask_sb[:])

            # ============ cat matmul (f16) + slim AG ============
            y_sb = [sm_p.tile([O, S], F32, name=f"y_sb{sd}") for sd in range(2)]
            stats = sm_p.tile([O, 4], F32, name="stats")

            for sd in range(2):
                ps_y = psmm.tile([O, S], F32, tag="mm", name="ps_y")
                for b in range(6):
                    nc.tensor.matmul(ps_y[:], wcat_sb[:, b, sd, :],
                                     catT[sd][:, b, :],
                                     start=(b == 0), stop=(b == 5))
                nc.vector.tensor_copy(y_sb[sd][:], ps_y[:])
                nc.vector.reduce_sum(stats[:, sd:sd + 1], y_sb[sd][:],
                                     axis=AXX)
                nc.vector.tensor_mul(junk[0:O, :], y_sb[sd][:],
                                     y_sb[sd][:])
                nc.vector.reduce_sum(stats[:, 2 + sd:3 + sd],
                                     junk[0:O, :], axis=AXX)
            ag_st = sm_p.tile([O, S + 4], F32, name="ag_st")
            nc.vector.tensor_copy(ag_st[:, 0:S], y_sb[1][:])
            nc.vector.tensor_copy(ag_st[:, S:S + 4], stats[:])
            nc.sync.dma_start(ag_in[:], ag_st[:])
            nc.gpsimd.collective_compute("AllGather", ALU.bypass,
                                         replica_groups=replica,
                                         ins=[ag_in.opt()],
                                         outs=[ag_out.opt()])

            yv_all = sm_p.tile([O, NC, S + 4], F32, name="yv_all")
            nc.sync.dma_start(yv_all[:, :, S:S + 4],
                              ag_out[:, :, S:S + 4].rearrange("c p x -> p c x"))
            nc.sync.dma_start(yv_all[:, 0:4, 0:S],
                              ag_out[0:4, :, 0:S].rearrange("c p x -> p c x"))
            nc.sync.dma_start(yv_all[:, 4:8, 0:S],
                              ag_out[4:8, :, 0:S].rearrange("c p x -> p c x"))

            # ============ cat BN (global stats) + embeds ============
            statacc = sm_p.tile([O, 4], F32, name="statacc")
            nc.vector.tensor_copy(statacc[:], yv_all[:, 0, S:S + 4])
            for c in range(1, NC):
                nc.vector.tensor_add(statacc[:], statacc[:],
                                     yv_all[:, c, S:S + 4])
            embed_u = sm_p.tile([O, S], F16)
            embed_v = sm_p.tile([O, UP], F16)
            scc, shc = bn_from_sums("cB", statacc[:, 0:2], statacc[:, 2:4],
                                    gbc_sb[:, 0:2], gbc_sb[:, 2:4], U, O)
            nc.scalar.activation(embed_u[:], y_sb[0][:], AF.Relu,
                                 bias=shc[:, 0:1], scale=scc[:, 0:1])
            ev = embed_v.rearrange("p (c u) -> p c u", c=NC)
            for hf in range(2):
                nc.scalar.activation(ev[:, 4 * hf:4 * hf + 4, :],
                                     yv_all[:, 4 * hf:4 * hf + 4, 0:S],
                                     AF.Relu, bias=shc[:, 1:2],
                                     scale=scc[:, 1:2])

            # ============ score ============
            for r in range(R):
                ps_t1 = psmm.tile([O, S], F32, tag="mm", name="ps_t1")
                nc.tensor.matmul(ps_t1[:], q_sb[:, r, :], embed_u[:],
                                 start=True, stop=True)
                t1 = sc_p.tile([O, S], F16, tag="t1", name="t1")
                nc.vector.tensor_copy(t1[:], ps_t1[:])
                for ch in range(2):
                    out_sb = sc_p.tile([128, V], F16, tag="osb", name="out_sb")
                    for i, (n0, nn) in enumerate(NTILES):
                        ps_sc = pssc.tile([128, 512], F32, tag="sc",
                                          name="ps_sc")
                        nc.tensor.matmul(ps_sc[:, 0:nn],
                                         t1[:, ch * 128:(ch + 1) * 128],
                                         embed_v[:, n0:n0 + nn],
                                         start=True, stop=True)
                        if i % 2 == 0:
                            nc.vector.tensor_copy(out_sb[:, n0:n0 + nn],
                                                  ps_sc[:, 0:nn])
                        else:
                            nc.scalar.copy(out_sb[:, n0:n0 + nn],
                                           ps_sc[:, 0:nn])
                    nc.sync.dma_start(score_d[r, ch * 128:(ch + 1) * 128, :],
                                      out_sb[:])

    nc.compile()
    return nc


def _prep(inputs):
    """Host-side shard/pad/scale/cast/pack. Returns in_maps for 8 cores."""
    def padto(a, n, axis):
        pad = [(0, 0)] * a.ndim
        pad[axis] = (0, n - a.shape[axis])
        return np.pad(a, pad)

    f32 = np.float32
    fu = padto(padto(np.asarray(inputs['feature_u'], f32), UP, 0), UP, 1)
    fv = padto(padto(np.asarray(inputs['feature_v'], f32), UP, 0), UP, 1)
    Mu = padto(padto(np.asarray(inputs['M_u'], f32), UP, 1), UP, 2) * M_SC
    Mv = padto(padto(np.asarray(inputs['M_v'], f32), UP, 1), UP, 2) * M_SC
    W = padto(np.asarray(inputs['W'], f32), UP, 1)
    sfu = padto(np.asarray(inputs['side_feature_u'], f32), UP, 0)
    sfv = padto(np.asarray(inputs['side_feature_v'], f32), UP, 0)

    # catT row order [pre | hidden | side]; reference cat order is
    # [hidden | f@W | side] -> permute w_cat rows to match.
    perm = np.concatenate([np.arange(RH, 2 * RH), np.arange(0, RH),
                           np.arange(2 * RH, 2 * RH + SH)])
    wcat = np.stack(
        [padto(np.asarray(inputs[f'w_cat_{s}'], f32)[perm], 6 * 128, 0)
         for s in ('u', 'v')], 1)                   # [768, 2, 75]
    wcat16 = np.ascontiguousarray(
        wcat.reshape(6, 128, 2, O).transpose(1, 0, 2, 3)).astype(F16N)
    wside = np.stack([np.asarray(inputs['w_side_u'], f32),
                      np.asarray(inputs['w_side_v'], f32)], 1).astype(F16N)
    gbs = np.stack([inputs['g_side_u'], inputs['g_side_v'],
                    inputs['beta_side_u'], inputs['beta_side_v']],
                   1).astype(f32)
    gbc = np.stack([inputs['g_cat_u'], inputs['g_cat_v'],
                    inputs['beta_cat_u'], inputs['beta_cat_v']],
                   1).astype(f32)
    # W repacked to [p, k, r*h] so each k-slice is a contiguous [128, RH] rhs
    w16 = np.ascontiguousarray(
        W.reshape(R, KT, 128, H).transpose(2, 1, 0, 3)).reshape(
        128, KT, RH).astype(F16N)
    q16 = np.ascontiguousarray(
        np.asarray(inputs['Q'], f32).transpose(1, 0, 2)).astype(F16N)

    def pack_f(feat, sl):
        # [2048, 256] rows sl -> [128, 16, 256]: f = k*128 + p
        a = np.ascontiguousarray(feat[sl].T)        # [2048 f, 256]
        return np.ascontiguousarray(
            a.reshape(KT, 128, S).transpose(1, 0, 2)).astype(F16N)

    def pack_m(Msc, r, sl):
        # M[r][:, my rows].T -> [128, 2, 2048]: local row = t*128 + p
        a = np.ascontiguousarray(Msc[r][:, sl].T)   # [256 local, 2048]
        return np.ascontiguousarray(
            a.reshape(2, 128, UP).transpose(1, 0, 2)).astype(F8N)

    in_maps = []
    for c in range(NC):
        sl = slice(c * S, (c + 1) * S)
        in_maps.append({
            "fuT": pack_f(fu, sl),
            "fvT": pack_f(fv, sl),
            "muT": np.stack([pack_m(Mu, r, sl) for r in range(R)]),
            "mvT": np.stack([pack_m(Mv, r, sl) for r in range(R)]),
            "w": w16,
            "q": q16,
            "sfT": np.ascontiguousarray(
                np.stack([sfu[sl].T, sfv[sl].T], 1)).astype(F16N),
            "wside": wside,
            "wcat": wcat16,
            "gb_side": gbs,
            "gb_cat": gbc,
            "ident": np.eye(128, dtype=F16N),
            "mask": np.broadcast_to(
                (np.arange(c * S, (c + 1) * S) < U).astype(F16N),
                (SH, S)).copy(),
        })
    return in_maps


def kernel(**inputs) -> np.ndarray:
    if "nc" not in _CACHE:
        _CACHE["nc"] = _build()
    nc = _CACHE["nc"]
    in_maps = _prep(inputs)
    res = bass_utils.run_bass_kernel_spmd(nc, in_maps, core_ids=list(range(NC)))
    score = np.concatenate(
        [np.asarray(res.results[c]["score"]) for c in range(NC)],
        axis=1).astype(np.float32)
    return score[:, :U, :]


if __name__ == "__main__":
    print("kernel module OK")


# revision 47
# speedup vs baseline: 1.0071x; 1.0071x over previous
"""Trainium2 Bass kernel for nn_GCMC (GNN message passing / GCMC scoring).

v5 strategy: row-shard users AND items across 8 NeuronCores (256 padded rows
each). Message passing is ONE merged ReduceScatter: each core column-shards
M (its 256 v-columns of M_u, u-columns of M_v), computes partial hidden sums
for ALL opposite-side rows from its local projection slice, and a single
ReduceScatter (add) over a [NC, 2, H, R, 258] fp16 payload returns both
sides' per-core hidden rows (side-branch BatchNorm partial sums ride in 2
extra columns). One slim AllGather then shares the pre-BN v-side cat output
y_v plus both sides' cat BatchNorm partial sums, so every core computes
global BatchNorm stats locally and the full embed_v for the final bilinear
score.

Precision: fp16 operands (the PE accumulates in fp32; fp16 measured no worse
than bf16 here), fp32 catT is not needed (fp16 catT/w_cat measured fine), M
and the projection copy it contracts with travel as fp8e4m3 (x64 / x4
scales, undone by the hidden-relu activation scale 2^-8) enabling DoubleRow
matmuls and halving the dominant M DMA traffic. The RS payload is fp16; the
AllGather payload MUST stay fp32 - 16-bit AllGather payloads measurably
degrade (~1.5% extra score error, consistent with the collective
round-tripping 16-bit data through bf16), while the fp16 ReduceScatter only
costs ~0.35%. Measured end-to-end max-rel error: 1.41% vs the 2% gate.

Collectives: 1x fp16 ReduceScatter (23.3us) + 1x fp32 AllGather (30.6us) on
the serial collective device, vs 2 RS + 1 AG = 77us in v2.1. A dummy Sqrt
activation at t=0 preloads the activation-function table so the BatchNorm
Sqrt does not pay a 1.3us table load on the post-RS critical path.
"""
import sys
if '/opt/trn_rl_repo' not in sys.path:
    sys.path.insert(0, '/opt/trn_rl_repo')

import numpy as np
import ml_dtypes

import concourse.bass as bass
import concourse.bacc as bacc
import concourse.mybir as mybir
import concourse.tile as tile
from concourse import bass_utils

F16N = np.float16
F8N = ml_dtypes.float8_e4m3
F32 = mybir.dt.float32
F16 = mybir.dt.float16
FP8 = mybir.dt.float8e4
AF = mybir.ActivationFunctionType
ALU = mybir.AluOpType
AXX = mybir.AxisListType.X
DR = mybir.MatmulPerfMode.DoubleRow

U = V = F = 2000
R, H, O, SH, SF = 5, 64, 75, 64, 128
RH = R * H           # 320
UP = 2048            # padded U/V/F
S = 256              # rows per core
SP2 = S + 2          # RS payload row width (256 data + 2 BN-sum cols)
NC = 8
KT = 16              # 128-row k-tiles over the padded 2048 contraction dims
EPS = 1e-5
M_SC, P_SC = 64.0, 4.0          # fp8 scales for M and prevh
HID_SC = 1.0 / (M_SC * P_SC)    # 2^-8, folded into hidden relu
NTILES = [(0, 512), (512, 512), (1024, 512), (1536, 464)]  # score v-tiles

_CACHE = {}


def _build():
    nc = bacc.Bacc("TRN2", target_bir_lowering=False, debug=False,
                   num_devices=NC)

    def din(name, shape, dt):
        return nc.dram_tensor(name, list(shape), dt, kind="ExternalInput").ap()

    fuT_d = din("fuT", (128, KT, S), F16)     # my u rows, [f, kt, u]
    fvT_d = din("fvT", (128, KT, S), F16)
    muT_d = din("muT", (R, 128, 2, UP), FP8)  # x64 M_u[r][:, my_v].T packed
    mvT_d = din("mvT", (R, 128, 2, UP), FP8)
    w_d = din("w", (128, KT, RH), F16)
    q_d = din("q", (O, R, O), F16)
    sfT_d = din("sfT", (SF, 2, S), F16)
    wside_d = din("wside", (SF, 2, SH), F16)
    wcat_d = din("wcat", (128, 6, 2, O), F16)  # rows: pre|hidden|side
    gbs_d = din("gb_side", (SH, 4), F32)
    gbc_d = din("gb_cat", (O, 4), F32)
    ident_d = din("ident", (128, 128), F16)
    mask_d = din("mask", (SH, S), F16)

    score_d = nc.dram_tensor("score", [R, S, V], F16,
                             kind="ExternalOutput").ap()

    with tile.TileContext(nc) as tc:
        with tc.tile_pool(name="const", bufs=1) as const_p, \
             tc.tile_pool(name="big", bufs=1) as big_p, \
             tc.tile_pool(name="mring", bufs=10) as m_p, \
             tc.tile_pool(name="small", bufs=1) as sm_p, \
             tc.tile_pool(name="scoresb", bufs=3) as sc_p, \
             tc.tile_pool(name="psmm", bufs=4, space="PSUM") as psmm, \
             tc.tile_pool(name="pssc", bufs=3, space="PSUM") as pssc, \
             tc.tile_pool(name="dram", bufs=1, space="DRAM") as dram_p:

            # ============ constant/small loads (SP queue) ============
            ident = const_p.tile([128, 128], F16)
            nc.sync.dma_start(ident[:], ident_d)
            eps_t = const_p.tile([128, 1], F32)
            nc.vector.memset(eps_t[:], EPS)
            sqrt_warm = const_p.tile([128, 1], F32, name="sqrt_warm")
            nc.scalar.activation(sqrt_warm[:], eps_t[:], AF.Sqrt,
                                 bias=eps_t[:])
            sfT_sb = const_p.tile([SF, 2, S], F16)
            nc.sync.dma_start(sfT_sb[:], sfT_d)
            wside_sb = const_p.tile([SF, 2, SH], F16)
            nc.sync.dma_start(wside_sb[:], wside_d)
            q_sb = const_p.tile([O, R, O], F16)
            nc.sync.dma_start(q_sb[:], q_d)
            wcat_sb = const_p.tile([128, 6, 2, O], F16)
            nc.sync.dma_start(wcat_sb[:], wcat_d)
            gbs_sb = const_p.tile([SH, 4], F32)
            nc.sync.dma_start(gbs_sb[:], gbs_d)
            gbc_sb = const_p.tile([O, 4], F32)
            nc.sync.dma_start(gbc_sb[:], gbc_d)
            mask_sb = const_p.tile([SH, S], F16)
            nc.sync.dma_start(mask_sb[:], mask_d)

            # ============ big stream (ACT queue, exact order) ============
            fvT_sb = big_p.tile([128, KT, S], F16)
            nc.scalar.dma_start(fvT_sb[:], fvT_d)
            w_sb = big_p.tile([128, KT, RH], F16)
            nc.scalar.dma_start(w_sb[:], w_d)
            fuT_sb = big_p.tile([128, KT, S], F16)
            nc.scalar.dma_start(fuT_sb[:], fuT_d)
            muT_sb = [m_p.tile([128, 2, UP], FP8, tag="mT", name=f"muT_{r}")
                      for r in range(R)]
            mvT_sb = [m_p.tile([128, 2, UP], FP8, tag="mT", name=f"mvT_{r}")
                      for r in range(R)]
            for r in range(R):
                nc.scalar.dma_start(muT_sb[r][:], muT_d[r])
            for r in range(R):
                nc.scalar.dma_start(mvT_sb[r][:], mvT_d[r])

            # ============ collective buffers ============
            replica = [list(range(NC))]
            rs_in = dram_p.tile([NC, 2, H, R, SP2], F16)
            rs_out = dram_p.tile([2, H, R, SP2], F16)
            ag_in = dram_p.tile([O, S + 4], F32)
            ag_out = dram_p.tile([NC, O, S + 4], F32, addr_space="Shared")

            # catT: [128, 6, S] f16 per side; rows pre(0:320)|hidden|side
            catT = [big_p.tile([128, 6, S], F16, name=f"catT{sd}")
                    for sd in range(2)]
            for sd in range(2):
                nc.vector.memset(catT[sd][SH:128, 5, :], 0.0)

            # partial-hidden staging, shared by both sides via WAR reuse
            stage = big_p.tile([H, NC, R, SP2], F16, name="stage")
            nc.vector.memset(stage[:, :, :, S:SP2], 0.0)

            # ============ side matmuls + BN partial sums ============
            s_loc = sm_p.tile([SH, 2, S], F32)
            junk = sm_p.tile([128, S], F32, name="junk")

            def side_branch(sd):
                ps_s = psmm.tile([SH, S], F32, tag="mm", name="ps_side")
                nc.tensor.matmul(ps_s[:], wside_sb[:, sd, :], sfT_sb[:, sd, :],
                                 start=True, stop=True)
                nc.vector.tensor_copy(s_loc[:, sd, :], ps_s[:])
                s_sums = sm_p.tile([SH, 2], F32, tag=f"s_sums{sd}",
                                   name=f"s_sums{sd}")
                nc.vector.reduce_sum(s_sums[:, 0:1], s_loc[:, sd, :], axis=AXX)
                nc.vector.tensor_mul(junk[0:SH, :], s_loc[:, sd, :],
                                     s_loc[:, sd, :])
                nc.vector.reduce_sum(s_sums[:, 1:2], junk[0:SH, :], axis=AXX)
                # replicate into every dest shard of the RS payload (row r=0)
                for c in range(NC):
                    nc.vector.tensor_copy(stage[:, c, 0, S:S + 2], s_sums[:])

            # ============ projection: pre[row, rh] = f^T W ============
            prevh = [big_p.tile([128, 2, RH], F16, name=f"prevh_{sd}")
                     for sd in range(2)]
            prevh8 = [big_p.tile([128, 2, RH], FP8, name=f"prevh8_{sd}")
                      for sd in range(2)]

            def proj_side(sd, fT):
                pre_f16 = prevh[sd]
                for ch in range(2):
                    ps_pre = psmm.tile([128, RH], F32, tag="mm", name="ps_pre")
                    for k in range(KT):
                        nc.tensor.matmul(ps_pre[:],
                                         fT[:, k, ch * 128:(ch + 1) * 128],
                                         w_sb[:, k, :],
                                         start=(k == 0), stop=(k == KT - 1))
                    nc.vector.tensor_copy(pre_f16[:, ch, :], ps_pre[:])
                    nc.scalar.activation(prevh8[sd][:, ch, :], ps_pre[:],
                                         AF.Copy, scale=P_SC)

            # deferred: transpose prevh into catT pre rows during the RS
            # window (keeps these copies off the pre-RS DVE critical path)
            def catT_pre_fill(sd):
                for ch in range(2):
                    for c in range(3):
                        cw = min(128, RH - c * 128)
                        ps_t = psmm.tile([128, 128], F16, tag="mm",
                                         name="ps_t")
                        nc.tensor.transpose(
                            ps_t[0:cw, :],
                            prevh[sd][:, ch, c * 128:c * 128 + cw],
                            ident[:])
                        row = c * 128
                        blk, off = divmod(row, 128)
                        eng = nc.vector if (ch + c) % 2 == 0 else None
                        if eng is None:
                            nc.scalar.copy(
                                catT[sd][off:off + cw, blk,
                                         ch * 128:(ch + 1) * 128],
                                ps_t[0:cw, :])
                        else:
                            nc.vector.tensor_copy(
                                catT[sd][off:off + cw, blk,
                                         ch * 128:(ch + 1) * 128],
                                ps_t[0:cw, :])

            # ==== partial hidden (fp8 DoubleRow) -> f16 stage -> DMA ====
            def partial_side(sd, mT, osd):
                for r in range(R):
                    pss = [psmm.tile([H, 2, S], F32, tag="mm", name=f"ps_p{g}")
                           for g in range(4)]
                    for g in range(4):
                        for c in range(2):
                            nc.tensor.matmul(
                                pss[g][:, c, :],
                                prevh8[osd][:, :, r * H:(r + 1) * H],
                                mT[r][:, :, (2 * g + c) * S:(2 * g + c + 1) * S],
                                start=True, stop=True, perf_mode=DR)
                    for g in range(4):
                        dst = stage[:, 2 * g:2 * g + 2, r, 0:S]
                        if g % 2 == 0:
                            nc.vector.tensor_copy(dst, pss[g][:])
                        else:
                            nc.scalar.copy(dst, pss[g][:])
                    nc.sync.dma_start(
                        rs_in[:, sd, :, r, :].rearrange("c h x -> h c x"),
                        stage[:, :, r, :])

            side_branch(0)
            proj_side(1, fvT_sb)
            partial_side(0, muT_sb, 1)
            side_branch(1)
            proj_side(0, fuT_sb)
            partial_side(1, mvT_sb, 0)
            catT_pre_fill(1)
            catT_pre_fill(0)
            nc.gpsimd.collective_compute("ReduceScatter", ALU.add,
                                         replica_groups=replica,
                                         ins=[rs_in.opt()],
                                         outs=[rs_out.opt()])



            # ============ BN helpers (both sides batched: [P, 2]) ======
            def bn_from_sums(tg, sums, sumsq, g_col, b_col, n, P):
                def t(nm):
                    return sm_p.tile([P, 2], F32, tag=f"{nm}_{tg}",
                                     name=f"{nm}_{tg}")
                mu = t("bn_mu")
                nc.vector.tensor_scalar_mul(mu[:], sums[:], 1.0 / n)
                e2 = t("bn_e2")
                nc.vector.tensor_scalar_mul(e2[:], sumsq[:], 1.0 / n)
                var = t("bn_var")
                nc.vector.tensor_mul(var[:], mu[:], mu[:])
                nc.vector.tensor_sub(var[:], e2[:], var[:])
                std = t("bn_std")
                nc.scalar.activation(std[:], var[:], AF.Sqrt, bias=eps_t[0:P, :])
                rstd = t("bn_rstd")
                nc.vector.reciprocal(rstd[:], std[:])
                scale = t("bn_scale")
                nc.vector.tensor_mul(scale[:], g_col, rstd[:])
                shift = t("bn_shift")
                nc.vector.tensor_mul(shift[:], mu[:], scale[:])
                nc.vector.tensor_sub(shift[:], b_col, shift[:])
                return scale, shift

            # ====== hidden relu into catT rows 320:640; side BN ======
            side_tmp = sm_p.tile([SH, 2, S], F16, name="side_tmp")
            hsum = sm_p.tile([H, 2, R, SP2], F16, name="hsum")
            nc.sync.dma_start(hsum[:], rs_out.rearrange("s h r x -> h s r x"))
            t_sums = sm_p.tile([SH, 2, 2], F32, name="t_sums")
            nc.vector.tensor_copy(t_sums[:], hsum[:, :, 0, S:S + 2])
            for sd in range(2):
                for r in range(R):
                    row = RH + r * H
                    blk, off = divmod(row, 128)
                    if r % 2 == 0:
                        nc.scalar.activation(catT[sd][off:off + H, blk, :],
                                             hsum[:, sd, r, 0:S],
                                             AF.Relu, scale=HID_SC)
                    else:
                        nc.vector.tensor_scalar(
                            catT[sd][off:off + H, blk, :], hsum[:, sd, r, 0:S],
                            HID_SC, 0.0, op0=ALU.mult, op1=ALU.max)
            sc2, sh2 = bn_from_sums("sB", t_sums[:, :, 0], t_sums[:, :, 1],
                                    gbs_sb[:, 0:2], gbs_sb[:, 2:4], U, SH)
            for sd in range(2):
                nc.scalar.activation(side_tmp[:, sd, :], s_loc[:, sd, :],
                                     AF.Relu, bias=sh2[:, sd:sd + 1],
                                     scale=sc2[:, sd:sd + 1])
                nc.vector.tensor_mul(catT[sd][0:SH, 5, :], side_tmp[:, sd, :],
                                     mask_sb[:])

            # ============ cat matmul (f16) + slim AG ============
            y_sb = [sm_p.tile([O, S], F32, name=f"y_sb{sd}") for sd in range(2)]
            stats = sm_p.tile([O, 4], F32, name="stats")

            for sd in range(2):
                ps_y = psmm.tile([O, S], F32, tag="mm", name="ps_y")
                for b in range(6):
                    nc.tensor.matmul(ps_y[:], wcat_sb[:, b, sd, :],
                                     catT[sd][:, b, :],
                                     start=(b == 0), stop=(b == 5))
                nc.vector.tensor_copy(y_sb[sd][:], ps_y[:])
                nc.vector.reduce_sum(stats[:, sd:sd + 1], y_sb[sd][:],
                                     axis=AXX)
                nc.vector.tensor_mul(junk[0:O, :], y_sb[sd][:],
                                     y_sb[sd][:])
                nc.vector.reduce_sum(stats[:, 2 + sd:3 + sd],
                                     junk[0:O, :], axis=AXX)
            ag_st = sm_p.tile([O, S + 4], F32, name="ag_st")
            nc.vector.tensor_copy(ag_st[:, 0:S], y_sb[1][:])
            nc.vector.tensor_copy(ag_st[:, S:S + 4], stats[:])
            nc.sync.dma_start(ag_in[:], ag_st[:])
            nc.gpsimd.collective_compute("AllGather", ALU.bypass,
                                         replica_groups=replica,
                                         ins=[ag_in.opt()],
                                         outs=[ag_out.opt()])

            yv_all = sm_p.tile([O, NC, S + 4], F32, name="yv_all")
            nc.sync.dma_start(yv_all[:, :, S:S + 4],
                              ag_out[:, :, S:S + 4].rearrange("c p x -> p c x"))
            nc.sync.dma_start(yv_all[:, 0:4, 0:S],
                              ag_out[0:4, :, 0:S].rearrange("c p x -> p c x"))
            nc.sync.dma_start(yv_all[:, 4:8, 0:S],
                              ag_out[4:8, :, 0:S].rearrange("c p x -> p c x"))

            # ============ cat BN (global stats) + embeds ============
            statacc = sm_p.tile([O, 4], F32, name="statacc")
            nc.vector.tensor_copy(statacc[:], yv_all[:, 0, S:S + 4])
            for c in range(1, NC):
                nc.vector.tensor_add(statacc[:], statacc[:],
                                     yv_all[:, c, S:S + 4])
            embed_u = sm_p.tile([O, S], F16)
            embed_v = sm_p.tile([O, UP], F16)
            scc, shc = bn_from_sums("cB", statacc[:, 0:2], statacc[:, 2:4],
                                    gbc_sb[:, 0:2], gbc_sb[:, 2:4], U, O)
            nc.scalar.activation(embed_u[:], y_sb[0][:], AF.Relu,
                                 bias=shc[:, 0:1], scale=scc[:, 0:1])
            ev = embed_v.rearrange("p (c u) -> p c u", c=NC)
            for hf in range(2):
                nc.scalar.activation(ev[:, 4 * hf:4 * hf + 4, :],
                                     yv_all[:, 4 * hf:4 * hf + 4, 0:S],
                                     AF.Relu, bias=shc[:, 1:2],
                                     scale=scc[:, 1:2])

            # ============ score ============
            for r in range(R):
                ps_t1 = psmm.tile([O, S], F32, tag="mm", name="ps_t1")
                nc.tensor.matmul(ps_t1[:], q_sb[:, r, :], embed_u[:],
                                 start=True, stop=True)
                t1 = sc_p.tile([O, S], F16, tag="t1", name="t1")
                nc.vector.tensor_copy(t1[:], ps_t1[:])
                for ch in range(2):
                    out_sb = sc_p.tile([128, V], F16, tag="osb", name="out_sb")
                    for i, (n0, nn) in enumerate(NTILES):
                        ps_sc = pssc.tile([128, 512], F32, tag="sc",
                                          name="ps_sc")
                        nc.tensor.matmul(ps_sc[:, 0:nn],
                                         t1[:, ch * 128:(ch + 1) * 128],
                                         embed_v[:, n0:n0 + nn],
                                         start=True, stop=True)
                        if i % 2 == 0:
                            nc.vector.tensor_copy(out_sb[:, n0:n0 + nn],
                                                  ps_sc[:, 0:nn])
                        else:
                            nc.scalar.copy(out_sb[:, n0:n0 + nn],
                                           ps_sc[:, 0:nn])
                    nc.sync.dma_start(score_d[r, ch * 128:(ch + 1) * 128, :],
                                      out_sb[:])

    nc.compile()
    return nc


def _prep(inputs):
    """Host-side shard/pad/scale/cast/pack. Returns in_maps for 8 cores."""
    def padto(a, n, axis):
        pad = [(0, 0)] * a.ndim
        pad[axis] = (0, n - a.shape[axis])
        return np.pad(a, pad)

    f32 = np.float32
    fu = padto(padto(np.asarray(inputs['feature_u'], f32), UP, 0), UP, 1)
    fv = padto(padto(np.asarray(inputs['feature_v'], f32), UP, 0), UP, 1)
    Mu = padto(padto(np.asarray(inputs['M_u'], f32), UP, 1), UP, 2) * M_SC
    Mv = padto(padto(np.asarray(inputs['M_v'], f32), UP, 1), UP, 2) * M_SC
    W = padto(np.asarray(inputs['W'], f32), UP, 1)
    sfu = padto(np.asarray(inputs['side_feature_u'], f32), UP, 0)
    sfv = padto(np.asarray(inputs['side_feature_v'], f32), UP, 0)

    # catT row order [pre | hidden | side]; reference cat order is
    # [hidden | f@W | side] -> permute w_cat rows to match.
    perm = np.concatenate([np.arange(RH, 2 * RH), np.arange(0, RH),
                           np.arange(2 * RH, 2 * RH + SH)])
    wcat = np.stack(
        [padto(np.asarray(inputs[f'w_cat_{s}'], f32)[perm], 6 * 128, 0)
         for s in ('u', 'v')], 1)                   # [768, 2, 75]
    wcat16 = np.ascontiguousarray(
        wcat.reshape(6, 128, 2, O).transpose(1, 0, 2, 3)).astype(F16N)
    wside = np.stack([np.asarray(inputs['w_side_u'], f32),
                      np.asarray(inputs['w_side_v'], f32)], 1).astype(F16N)
    gbs = np.stack([inputs['g_side_u'], inputs['g_side_v'],
                    inputs['beta_side_u'], inputs['beta_side_v']],
                   1).astype(f32)
    gbc = np.stack([inputs['g_cat_u'], inputs['g_cat_v'],
                    inputs['beta_cat_u'], inputs['beta_cat_v']],
                   1).astype(f32)
    # W repacked to [p, k, r*h] so each k-slice is a contiguous [128, RH] rhs
    w16 = np.ascontiguousarray(
        W.reshape(R, KT, 128, H).transpose(2, 1, 0, 3)).reshape(
        128, KT, RH).astype(F16N)
    q16 = np.ascontiguousarray(
        np.asarray(inputs['Q'], f32).transpose(1, 0, 2)).astype(F16N)

    def pack_f(feat, sl):
        # [2048, 256] rows sl -> [128, 16, 256]: f = k*128 + p
        a = np.ascontiguousarray(feat[sl].T)        # [2048 f, 256]
        return np.ascontiguousarray(
            a.reshape(KT, 128, S).transpose(1, 0, 2)).astype(F16N)

    def pack_m(Msc, r, sl):
        # M[r][:, my rows].T -> [128, 2, 2048]: local row = t*128 + p
        a = np.ascontiguousarray(Msc[r][:, sl].T)   # [256 local, 2048]
        return np.ascontiguousarray(
            a.reshape(2, 128, UP).transpose(1, 0, 2)).astype(F8N)

    in_maps = []
    for c in range(NC):
        sl = slice(c * S, (c + 1) * S)
        in_maps.append({
            "fuT": pack_f(fu, sl),
            "fvT": pack_f(fv, sl),
            "muT": np.stack([pack_m(Mu, r, sl) for r in range(R)]),
            "mvT": np.stack([pack_m(Mv, r, sl) for r in range(R)]),
            "w": w16,
            "q": q16,
            "sfT": np.ascontiguousarray(
                np.stack([sfu[sl].T, sfv[sl].T], 1)).astype(F16N),
            "wside": wside,
            "wcat": wcat16,
            "gb_side": gbs,
            "gb_cat": gbc,
            "ident": np.eye(128, dtype=F16N),
            "mask": np.broadcast_to(
                (np.arange(c * S, (c + 1) * S) < U).astype(F16N),
                (SH, S)).copy(),
        })
    return in_maps


def kernel(**inputs) -> np.ndarray:
    if "nc" not in _CACHE:
        _CACHE["nc"] = _build()
    nc = _CACHE["nc"]
    in_maps = _prep(inputs)
    res = bass_utils.run_bass_kernel_spmd(nc, in_maps, core_ids=list(range(NC)))
    score = np.concatenate(
        [np.asarray(res.results[c]["score"]) for c in range(NC)],
        axis=1).astype(np.float32)
    return score[:, :U, :]


if __name__ == "__main__":
    print("kernel module OK")


# revision 48
# speedup vs baseline: 1.0119x; 1.0048x over previous
"""Trainium2 Bass kernel for nn_GCMC (GNN message passing / GCMC scoring).

v5 strategy: row-shard users AND items across 8 NeuronCores (256 padded rows
each). Message passing is ONE merged ReduceScatter: each core column-shards
M (its 256 v-columns of M_u, u-columns of M_v), computes partial hidden sums
for ALL opposite-side rows from its local projection slice, and a single
ReduceScatter (add) over a [NC, 2, H, R, 258] fp16 payload returns both
sides' per-core hidden rows (side-branch BatchNorm partial sums ride in 2
extra columns). One slim AllGather then shares the pre-BN v-side cat output
y_v plus both sides' cat BatchNorm partial sums, so every core computes
global BatchNorm stats locally and the full embed_v for the final bilinear
score.

Precision: fp16 operands (the PE accumulates in fp32; fp16 measured no worse
than bf16 here), fp32 catT is not needed (fp16 catT/w_cat measured fine), M
and the projection copy it contracts with travel as fp8e4m3 (x64 / x4
scales, undone by the hidden-relu activation scale 2^-8) enabling DoubleRow
matmuls and halving the dominant M DMA traffic. The RS payload is fp16; the
AllGather payload MUST stay fp32 - 16-bit AllGather payloads measurably
degrade (~1.5% extra score error, consistent with the collective
round-tripping 16-bit data through bf16), while the fp16 ReduceScatter only
costs ~0.35%. Measured end-to-end max-rel error: 1.41% vs the 2% gate.

Collectives: 1x fp16 ReduceScatter (23.3us) + 1x fp32 AllGather (30.6us) on
the serial collective device, vs 2 RS + 1 AG = 77us in v2.1. A dummy Sqrt
activation at t=0 preloads the activation-function table so the BatchNorm
Sqrt does not pay a 1.3us table load on the post-RS critical path.
"""
import sys
if '/opt/trn_rl_repo' not in sys.path:
    sys.path.insert(0, '/opt/trn_rl_repo')

import numpy as np
import ml_dtypes

import concourse.bass as bass
import concourse.bacc as bacc
import concourse.mybir as mybir
import concourse.tile as tile
from concourse import bass_utils

F16N = np.float16
F8N = ml_dtypes.float8_e4m3
F32 = mybir.dt.float32
F16 = mybir.dt.float16
FP8 = mybir.dt.float8e4
AF = mybir.ActivationFunctionType
ALU = mybir.AluOpType
AXX = mybir.AxisListType.X
DR = mybir.MatmulPerfMode.DoubleRow

U = V = F = 2000
R, H, O, SH, SF = 5, 64, 75, 64, 128
RH = R * H           # 320
UP = 2048            # padded U/V/F
S = 256              # rows per core
SP2 = S + 2          # RS payload row width (256 data + 2 BN-sum cols)
NC = 8
KT = 16              # 128-row k-tiles over the padded 2048 contraction dims
EPS = 1e-5
M_SC, P_SC = 64.0, 4.0          # fp8 scales for M and prevh
HID_SC = 1.0 / (M_SC * P_SC)    # 2^-8, folded into hidden relu
NTILES = [(0, 512), (512, 512), (1024, 512), (1536, 464)]  # score v-tiles

_CACHE = {}


def _build():
    nc = bacc.Bacc("TRN2", target_bir_lowering=False, debug=False,
                   num_devices=NC)

    def din(name, shape, dt):
        return nc.dram_tensor(name, list(shape), dt, kind="ExternalInput").ap()

    fuT_d = din("fuT", (128, KT, S), F16)     # my u rows, [f, kt, u]
    fvT_d = din("fvT", (128, KT, S), F16)
    muT_d = din("muT", (R, 128, 2, UP), FP8)  # x64 M_u[r][:, my_v].T packed
    mvT_d = din("mvT", (R, 128, 2, UP), FP8)
    w_d = din("w", (128, KT, RH), F16)
    q_d = din("q", (O, R, O), F16)
    sfT_d = din("sfT", (SF, 2, S), F16)
    wside_d = din("wside", (SF, 2, SH), F16)
    wcat_d = din("wcat", (128, 6, 2, O), F16)  # rows: pre|hidden|side
    gbs_d = din("gb_side", (SH, 4), F32)
    gbc_d = din("gb_cat", (O, 4), F32)
    ident_d = din("ident", (128, 128), F16)
    mask_d = din("mask", (SH, S), F16)

    score_d = nc.dram_tensor("score", [R, S, V], F16,
                             kind="ExternalOutput").ap()

    with tile.TileContext(nc) as tc:
        with tc.tile_pool(name="const", bufs=1) as const_p, \
             tc.tile_pool(name="big", bufs=1) as big_p, \
             tc.tile_pool(name="mring", bufs=10) as m_p, \
             tc.tile_pool(name="small", bufs=1) as sm_p, \
             tc.tile_pool(name="scoresb", bufs=3) as sc_p, \
             tc.tile_pool(name="psmm", bufs=4, space="PSUM") as psmm, \
             tc.tile_pool(name="pssc", bufs=3, space="PSUM") as pssc, \
             tc.tile_pool(name="dram", bufs=1, space="DRAM") as dram_p:

            # ============ constant/small loads (SP queue) ============
            ident = const_p.tile([128, 128], F16)
            nc.sync.dma_start(ident[:], ident_d)
            eps_t = const_p.tile([128, 1], F32)
            nc.vector.memset(eps_t[:], EPS)
            sqrt_warm = const_p.tile([128, 1], F32, name="sqrt_warm")
            nc.scalar.activation(sqrt_warm[:], eps_t[:], AF.Sqrt,
                                 bias=eps_t[:])
            sfT_sb = const_p.tile([SF, 2, S], F16)
            nc.sync.dma_start(sfT_sb[:], sfT_d)
            wside_sb = const_p.tile([SF, 2, SH], F16)
            nc.sync.dma_start(wside_sb[:], wside_d)
            q_sb = const_p.tile([O, R, O], F16)
            nc.sync.dma_start(q_sb[:], q_d)
            wcat_sb = const_p.tile([128, 6, 2, O], F16)
            nc.sync.dma_start(wcat_sb[:], wcat_d)
            gbs_sb = const_p.tile([SH, 4], F32)
            nc.sync.dma_start(gbs_sb[:], gbs_d)
            gbc_sb = const_p.tile([O, 4], F32)
            nc.sync.dma_start(gbc_sb[:], gbc_d)
            mask_sb = const_p.tile([SH, S], F16)
            nc.sync.dma_start(mask_sb[:], mask_d)

            # ============ big stream (ACT queue, exact order) ============
            w_sb = big_p.tile([128, KT, RH], F16)
            nc.scalar.dma_start(w_sb[:], w_d)
            fvT_sb = big_p.tile([128, KT, S], F16)
            nc.scalar.dma_start(fvT_sb[:, :, 0:128], fvT_d[:, :, 0:128])
            nc.scalar.dma_start(fvT_sb[:, :, 128:S], fvT_d[:, :, 128:S])
            fuT_sb = big_p.tile([128, KT, S], F16)
            nc.scalar.dma_start(fuT_sb[:], fuT_d)
            muT_sb = [m_p.tile([128, 2, UP], FP8, tag="mT", name=f"muT_{r}")
                      for r in range(R)]
            mvT_sb = [m_p.tile([128, 2, UP], FP8, tag="mT", name=f"mvT_{r}")
                      for r in range(R)]
            for r in range(R):
                nc.scalar.dma_start(muT_sb[r][:], muT_d[r])
            for r in range(R):
                nc.scalar.dma_start(mvT_sb[r][:], mvT_d[r])

            # ============ collective buffers ============
            replica = [list(range(NC))]
            rs_in = dram_p.tile([NC, 2, H, R, SP2], F16)
            rs_out = dram_p.tile([2, H, R, SP2], F16)
            ag_in = dram_p.tile([O, S + 4], F32)
            ag_out = dram_p.tile([NC, O, S + 4], F32, addr_space="Shared")

            # catT: [128, 6, S] f16 per side; rows pre(0:320)|hidden|side
            catT = [big_p.tile([128, 6, S], F16, name=f"catT{sd}")
                    for sd in range(2)]
            for sd in range(2):
                nc.vector.memset(catT[sd][SH:128, 5, :], 0.0)

            # partial-hidden staging, shared by both sides via WAR reuse
            stage = big_p.tile([H, NC, R, SP2], F16, name="stage")
            nc.vector.memset(stage[:, :, :, S:SP2], 0.0)

            # ============ side matmuls + BN partial sums ============
            s_loc = sm_p.tile([SH, 2, S], F32)
            junk = sm_p.tile([128, S], F32, name="junk")

            s_sums = [sm_p.tile([SH, 2], F32, name=f"s_sums{sd}")
                      for sd in range(2)]

            def side_compute(sd):
                ps_s = psmm.tile([SH, S], F32, tag="mm", name="ps_side")
                nc.tensor.matmul(ps_s[:], wside_sb[:, sd, :], sfT_sb[:, sd, :],
                                 start=True, stop=True)
                nc.vector.tensor_copy(s_loc[:, sd, :], ps_s[:])
                nc.vector.reduce_sum(s_sums[sd][:, 0:1], s_loc[:, sd, :],
                                     axis=AXX)
                nc.vector.tensor_mul(junk[0:SH, :], s_loc[:, sd, :],
                                     s_loc[:, sd, :])
                nc.vector.reduce_sum(s_sums[sd][:, 1:2], junk[0:SH, :],
                                     axis=AXX)

            def side_stage(sd):
                # replicate into every dest shard of the RS payload (row r=0)
                for c in range(NC):
                    nc.vector.tensor_copy(stage[:, c, 0, S:S + 2],
                                          s_sums[sd][:])

            # ============ projection: pre[row, rh] = f^T W ============
            prevh = [big_p.tile([128, 2, RH], F16, name=f"prevh_{sd}")
                     for sd in range(2)]
            prevh8 = [big_p.tile([128, 2, RH], FP8, name=f"prevh8_{sd}")
                      for sd in range(2)]

            def proj_side(sd, fT):
                pre_f16 = prevh[sd]
                for ch in range(2):
                    ps_pre = psmm.tile([128, RH], F32, tag="mm", name="ps_pre")
                    for k in range(KT):
                        nc.tensor.matmul(ps_pre[:],
                                         fT[:, k, ch * 128:(ch + 1) * 128],
                                         w_sb[:, k, :],
                                         start=(k == 0), stop=(k == KT - 1))
                    nc.vector.tensor_copy(pre_f16[:, ch, :], ps_pre[:])
                    nc.scalar.activation(prevh8[sd][:, ch, :], ps_pre[:],
                                         AF.Copy, scale=P_SC)

            # deferred: transpose prevh into catT pre rows during the RS
            # window (keeps these copies off the pre-RS DVE critical path)
            def catT_pre_fill(sd):
                for ch in range(2):
                    for c in range(3):
                        cw = min(128, RH - c * 128)
                        ps_t = psmm.tile([128, 128], F16, tag="mm",
                                         name="ps_t")
                        nc.tensor.transpose(
                            ps_t[0:cw, :],
                            prevh[sd][:, ch, c * 128:c * 128 + cw],
                            ident[:])
                        row = c * 128
                        blk, off = divmod(row, 128)
                        eng = nc.vector if (ch + c) % 2 == 0 else None
                        if eng is None:
                            nc.scalar.copy(
                                catT[sd][off:off + cw, blk,
                                         ch * 128:(ch + 1) * 128],
                                ps_t[0:cw, :])
                        else:
                            nc.vector.tensor_copy(
                                catT[sd][off:off + cw, blk,
                                         ch * 128:(ch + 1) * 128],
                                ps_t[0:cw, :])

            # ==== partial hidden (fp8 DoubleRow) -> f16 stage -> DMA ====
            def partial_side(sd, mT, osd):
                for r in range(R):
                    pss = [psmm.tile([H, 2, S], F32, tag="mm", name=f"ps_p{g}")
                           for g in range(4)]
                    for g in range(4):
                        for c in range(2):
                            nc.tensor.matmul(
                                pss[g][:, c, :],
                                prevh8[osd][:, :, r * H:(r + 1) * H],
                                mT[r][:, :, (2 * g + c) * S:(2 * g + c + 1) * S],
                                start=True, stop=True, perf_mode=DR)
                    for g in range(4):
                        dst = stage[:, 2 * g:2 * g + 2, r, 0:S]
                        if g % 2 == 0:
                            nc.vector.tensor_copy(dst, pss[g][:])
                        else:
                            nc.scalar.copy(dst, pss[g][:])
                    nc.sync.dma_start(
                        rs_in[:, sd, :, r, :].rearrange("c h x -> h c x"),
                        stage[:, :, r, :])

            side_compute(0)
            side_compute(1)
            side_stage(0)
            proj_side(1, fvT_sb)
            partial_side(0, muT_sb, 1)
            side_stage(1)
            proj_side(0, fuT_sb)
            partial_side(1, mvT_sb, 0)
            catT_pre_fill(1)
            catT_pre_fill(0)
            nc.gpsimd.collective_compute("ReduceScatter", ALU.add,
                                         replica_groups=replica,
                                         ins=[rs_in.opt()],
                                         outs=[rs_out.opt()])



            # ============ BN helpers (both sides batched: [P, 2]) ======
            def bn_from_sums(tg, sums, sumsq, g_col, b_col, n, P):
                def t(nm):
                    return sm_p.tile([P, 2], F32, tag=f"{nm}_{tg}",
                                     name=f"{nm}_{tg}")
                mu = t("bn_mu")
                nc.vector.tensor_scalar_mul(mu[:], sums[:], 1.0 / n)
                e2 = t("bn_e2")
                nc.vector.tensor_scalar_mul(e2[:], sumsq[:], 1.0 / n)
                var = t("bn_var")
                nc.vector.tensor_mul(var[:], mu[:], mu[:])
                nc.vector.tensor_sub(var[:], e2[:], var[:])
                std = t("bn_std")
                nc.scalar.activation(std[:], var[:], AF.Sqrt, bias=eps_t[0:P, :])
                rstd = t("bn_rstd")
                nc.vector.reciprocal(rstd[:], std[:])
                scale = t("bn_scale")
                nc.vector.tensor_mul(scale[:], g_col, rstd[:])
                shift = t("bn_shift")
                nc.vector.tensor_mul(shift[:], mu[:], scale[:])
                nc.vector.tensor_sub(shift[:], b_col, shift[:])
                return scale, shift

            # ====== hidden relu into catT rows 320:640; side BN ======
            side_tmp = sm_p.tile([SH, 2, S], F16, name="side_tmp")
            hsum = sm_p.tile([H, 2, R, SP2], F16, name="hsum")
            nc.sync.dma_start(hsum[:], rs_out.rearrange("s h r x -> h s r x"))
            t_sums = sm_p.tile([SH, 2, 2], F32, name="t_sums")
            nc.vector.tensor_copy(t_sums[:], hsum[:, :, 0, S:S + 2])
            for sd in range(2):
                for r in range(R):
                    row = RH + r * H
                    blk, off = divmod(row, 128)
                    if r % 2 == 0:
                        nc.scalar.activation(catT[sd][off:off + H, blk, :],
                                             hsum[:, sd, r, 0:S],
                                             AF.Relu, scale=HID_SC)
                    else:
                        nc.vector.tensor_scalar(
                            catT[sd][off:off + H, blk, :], hsum[:, sd, r, 0:S],
                            HID_SC, 0.0, op0=ALU.mult, op1=ALU.max)
            sc2, sh2 = bn_from_sums("sB", t_sums[:, :, 0], t_sums[:, :, 1],
                                    gbs_sb[:, 0:2], gbs_sb[:, 2:4], U, SH)
            for sd in range(2):
                nc.scalar.activation(side_tmp[:, sd, :], s_loc[:, sd, :],
                                     AF.Relu, bias=sh2[:, sd:sd + 1],
                                     scale=sc2[:, sd:sd + 1])
                nc.vector.tensor_mul(catT[sd][0:SH, 5, :], side_tmp[:, sd, :],
                                     mask_sb[:])

            # ============ cat matmul (f16) + slim AG ============
            y_sb = [sm_p.tile([O, S], F32, name=f"y_sb{sd}") for sd in range(2)]
            stats = sm_p.tile([O, 4], F32, name="stats")

            for sd in range(2):
                ps_y = psmm.tile([O, S], F32, tag="mm", name="ps_y")
                for b in range(6):
                    nc.tensor.matmul(ps_y[:], wcat_sb[:, b, sd, :],
                                     catT[sd][:, b, :],
                                     start=(b == 0), stop=(b == 5))
                nc.vector.tensor_copy(y_sb[sd][:], ps_y[:])
                nc.vector.reduce_sum(stats[:, sd:sd + 1], y_sb[sd][:],
                                     axis=AXX)
                nc.vector.tensor_mul(junk[0:O, :], y_sb[sd][:],
                                     y_sb[sd][:])
                nc.vector.reduce_sum(stats[:, 2 + sd:3 + sd],
                                     junk[0:O, :], axis=AXX)
            ag_st = sm_p.tile([O, S + 4], F32, name="ag_st")
            nc.vector.tensor_copy(ag_st[:, 0:S], y_sb[1][:])
            nc.vector.tensor_copy(ag_st[:, S:S + 4], stats[:])
            nc.sync.dma_start(ag_in[:], ag_st[:])
            nc.gpsimd.collective_compute("AllGather", ALU.bypass,
                                         replica_groups=replica,
                                         ins=[ag_in.opt()],
                                         outs=[ag_out.opt()])

            yv_all = sm_p.tile([O, NC, S + 4], F32, name="yv_all")
            nc.sync.dma_start(yv_all[:, :, S:S + 4],
                              ag_out[:, :, S:S + 4].rearrange("c p x -> p c x"))
            nc.sync.dma_start(yv_all[:, 0:4, 0:S],
                              ag_out[0:4, :, 0:S].rearrange("c p x -> p c x"))
            nc.sync.dma_start(yv_all[:, 4:8, 0:S],
                              ag_out[4:8, :, 0:S].rearrange("c p x -> p c x"))

            # ============ cat BN (global stats) + embeds ============
            statacc = sm_p.tile([O, 4], F32, name="statacc")
            nc.vector.tensor_copy(statacc[:], yv_all[:, 0, S:S + 4])
            for c in range(1, NC):
                nc.vector.tensor_add(statacc[:], statacc[:],
                                     yv_all[:, c, S:S + 4])
            embed_u = sm_p.tile([O, S], F16)
            embed_v = sm_p.tile([O, UP], F16)
            scc, shc = bn_from_sums("cB", statacc[:, 0:2], statacc[:, 2:4],
                                    gbc_sb[:, 0:2], gbc_sb[:, 2:4], U, O)
            nc.scalar.activation(embed_u[:], y_sb[0][:], AF.Relu,
                                 bias=shc[:, 0:1], scale=scc[:, 0:1])
            ev = embed_v.rearrange("p (c u) -> p c u", c=NC)
            for hf in range(2):
                nc.scalar.activation(ev[:, 4 * hf:4 * hf + 4, :],
                                     yv_all[:, 4 * hf:4 * hf + 4, 0:S],
                                     AF.Relu, bias=shc[:, 1:2],
                                     scale=scc[:, 1:2])

            # ============ score ============
            for r in range(R):
                ps_t1 = psmm.tile([O, S], F32, tag="mm", name="ps_t1")
                nc.tensor.matmul(ps_t1[:], q_sb[:, r, :], embed_u[:],
                                 start=True, stop=True)
                t1 = sc_p.tile([O, S], F16, tag="t1", name="t1")
                nc.vector.tensor_copy(t1[:], ps_t1[:])
                for ch in range(2):
                    out_sb = sc_p.tile([128, V], F16, tag="osb", name="out_sb")
                    for i, (n0, nn) in enumerate(NTILES):
                        ps_sc = pssc.tile([128, 512], F32, tag="sc",
                                          name="ps_sc")
                        nc.tensor.matmul(ps_sc[:, 0:nn],
                                         t1[:, ch * 128:(ch + 1) * 128],
                                         embed_v[:, n0:n0 + nn],
                                         start=True, stop=True)
                        if i % 2 == 0:
                            nc.vector.tensor_copy(out_sb[:, n0:n0 + nn],
                                                  ps_sc[:, 0:nn])
                        else:
                            nc.scalar.copy(out_sb[:, n0:n0 + nn],
                                           ps_sc[:, 0:nn])
                    nc.sync.dma_start(score_d[r, ch * 128:(ch + 1) * 128, :],
                                      out_sb[:])

    nc.compile()
    return nc


def _prep(inputs):
    """Host-side shard/pad/scale/cast/pack. Returns in_maps for 8 cores."""
    def padto(a, n, axis):
        pad = [(0, 0)] * a.ndim
        pad[axis] = (0, n - a.shape[axis])
        return np.pad(a, pad)

    f32 = np.float32
    fu = padto(padto(np.asarray(inputs['feature_u'], f32), UP, 0), UP, 1)
    fv = padto(padto(np.asarray(inputs['feature_v'], f32), UP, 0), UP, 1)
    Mu = padto(padto(np.asarray(inputs['M_u'], f32), UP, 1), UP, 2) * M_SC
    Mv = padto(padto(np.asarray(inputs['M_v'], f32), UP, 1), UP, 2) * M_SC
    W = padto(np.asarray(inputs['W'], f32), UP, 1)
    sfu = padto(np.asarray(inputs['side_feature_u'], f32), UP, 0)
    sfv = padto(np.asarray(inputs['side_feature_v'], f32), UP, 0)

    # catT row order [pre | hidden | side]; reference cat order is
    # [hidden | f@W | side] -> permute w_cat rows to match.
    perm = np.concatenate([np.arange(RH, 2 * RH), np.arange(0, RH),
                           np.arange(2 * RH, 2 * RH + SH)])
    wcat = np.stack(
        [padto(np.asarray(inputs[f'w_cat_{s}'], f32)[perm], 6 * 128, 0)
         for s in ('u', 'v')], 1)                   # [768, 2, 75]
    wcat16 = np.ascontiguousarray(
        wcat.reshape(6, 128, 2, O).transpose(1, 0, 2, 3)).astype(F16N)
    wside = np.stack([np.asarray(inputs['w_side_u'], f32),
                      np.asarray(inputs['w_side_v'], f32)], 1).astype(F16N)
    gbs = np.stack([inputs['g_side_u'], inputs['g_side_v'],
                    inputs['beta_side_u'], inputs['beta_side_v']],
                   1).astype(f32)
    gbc = np.stack([inputs['g_cat_u'], inputs['g_cat_v'],
                    inputs['beta_cat_u'], inputs['beta_cat_v']],
                   1).astype(f32)
    # W repacked to [p, k, r*h] so each k-slice is a contiguous [128, RH] rhs
    w16 = np.ascontiguousarray(
        W.reshape(R, KT, 128, H).transpose(2, 1, 0, 3)).reshape(
        128, KT, RH).astype(F16N)
    q16 = np.ascontiguousarray(
        np.asarray(inputs['Q'], f32).transpose(1, 0, 2)).astype(F16N)

    def pack_f(feat, sl):
        # [2048, 256] rows sl -> [128, 16, 256]: f = k*128 + p
        a = np.ascontiguousarray(feat[sl].T)        # [2048 f, 256]
        return np.ascontiguousarray(
            a.reshape(KT, 128, S).transpose(1, 0, 2)).astype(F16N)

    def pack_m(Msc, r, sl):
        # M[r][:, my rows].T -> [128, 2, 2048]: local row = t*128 + p
        a = np.ascontiguousarray(Msc[r][:, sl].T)   # [256 local, 2048]
        return np.ascontiguousarray(
            a.reshape(2, 128, UP).transpose(1, 0, 2)).astype(F8N)

    in_maps = []
    for c in range(NC):
        sl = slice(c * S, (c + 1) * S)
        in_maps.append({
            "fuT": pack_f(fu, sl),
            "fvT": pack_f(fv, sl),
            "muT": np.stack([pack_m(Mu, r, sl) for r in range(R)]),
            "mvT": np.stack([pack_m(Mv, r, sl) for r in range(R)]),
            "w": w16,
            "q": q16,
            "sfT": np.ascontiguousarray(
                np.stack([sfu[sl].T, sfv[sl].T], 1)).astype(F16N),
            "wside": wside,
            "wcat": wcat16,
            "gb_side": gbs,
            "gb_cat": gbc,
            "ident": np.eye(128, dtype=F16N),
            "mask": np.broadcast_to(
                (np.arange(c * S, (c + 1) * S) < U).astype(F16N),
                (SH, S)).copy(),
        })
    return in_maps


def kernel(**inputs) -> np.ndarray:
    if "nc" not in _CACHE:
        _CACHE["nc"] = _build()
    nc = _CACHE["nc"]
    in_maps = _prep(inputs)
    res = bass_utils.run_bass_kernel_spmd(nc, in_maps, core_ids=list(range(NC)))
    score = np.concatenate(
        [np.asarray(res.results[c]["score"]) for c in range(NC)],
        axis=1).astype(np.float32)
    return score[:, :U, :]


if __name__ == "__main__":
    print("kernel module OK")


# revision 49
# speedup vs baseline: 1.0252x; 1.0131x over previous
"""Trainium2 Bass kernel for nn_GCMC (GNN message passing / GCMC scoring).

v5 strategy: row-shard users AND items across 8 NeuronCores (256 padded rows
each). Message passing is ONE merged ReduceScatter: each core column-shards
M (its 256 v-columns of M_u, u-columns of M_v), computes partial hidden sums
for ALL opposite-side rows from its local projection slice, and a single
ReduceScatter (add) over a [NC, 2, H, R, 258] fp16 payload returns both
sides' per-core hidden rows (side-branch BatchNorm partial sums ride in 2
extra columns). One slim AllGather then shares the pre-BN v-side cat output
y_v plus both sides' cat BatchNorm partial sums, so every core computes
global BatchNorm stats locally and the full embed_v for the final bilinear
score.

Precision: fp16 operands (the PE accumulates in fp32; fp16 measured no worse
than bf16 here), fp32 catT is not needed (fp16 catT/w_cat measured fine), M
and the projection copy it contracts with travel as fp8e4m3 (x64 / x4
scales, undone by the hidden-relu activation scale 2^-8) enabling DoubleRow
matmuls and halving the dominant M DMA traffic. The RS payload is fp16; the
AllGather payload MUST stay fp32 - 16-bit AllGather payloads measurably
degrade (~1.5% extra score error, consistent with the collective
round-tripping 16-bit data through bf16), while the fp16 ReduceScatter only
costs ~0.35%. Measured end-to-end max-rel error: 1.41% vs the 2% gate.

Collectives: 1x fp16 ReduceScatter (23.3us) + 1x fp32 AllGather (30.6us) on
the serial collective device, vs 2 RS + 1 AG = 77us in v2.1. A dummy Sqrt
activation at t=0 preloads the activation-function table so the BatchNorm
Sqrt does not pay a 1.3us table load on the post-RS critical path.
"""
import sys
if '/opt/trn_rl_repo' not in sys.path:
    sys.path.insert(0, '/opt/trn_rl_repo')

import numpy as np
import ml_dtypes

import concourse.bass as bass
import concourse.bacc as bacc
import concourse.mybir as mybir
import concourse.tile as tile
from concourse import bass_utils

F16N = np.float16
F8N = ml_dtypes.float8_e4m3
F32 = mybir.dt.float32
F16 = mybir.dt.float16
FP8 = mybir.dt.float8e4
AF = mybir.ActivationFunctionType
ALU = mybir.AluOpType
AXX = mybir.AxisListType.X
DR = mybir.MatmulPerfMode.DoubleRow

U = V = F = 2000
R, H, O, SH, SF = 5, 64, 75, 64, 128
RH = R * H           # 320
UP = 2048            # padded U/V/F
S = 256              # rows per core
SP2 = S + 2          # RS payload row width (256 data + 2 BN-sum cols)
NC = 8
KT = 16              # 128-row k-tiles over the padded 2048 contraction dims
EPS = 1e-5
M_SC, P_SC = 64.0, 4.0          # fp8 scales for M and prevh
HID_SC = 1.0 / (M_SC * P_SC)    # 2^-8, folded into hidden relu
NTILES = [(0, 512), (512, 512), (1024, 512), (1536, 464)]  # score v-tiles

_CACHE = {}


def _build():
    nc = bacc.Bacc("TRN2", target_bir_lowering=False, debug=False,
                   num_devices=NC)

    def din(name, shape, dt):
        return nc.dram_tensor(name, list(shape), dt, kind="ExternalInput").ap()

    fuT_d = din("fuT", (128, KT, S), F16)     # my u rows, [f, kt, u]
    fvT_d = din("fvT", (128, KT, S), F16)
    muT_d = din("muT", (R, 128, 2, UP), FP8)  # x64 M_u[r][:, my_v].T packed
    mvT_d = din("mvT", (R, 128, 2, UP), FP8)
    w_d = din("w", (128, KT, RH), F16)
    q_d = din("q", (O, R, O), F16)
    sfT_d = din("sfT", (SF, 2, S), F16)
    wside_d = din("wside", (SF, 2, SH), F16)
    wcat_d = din("wcat", (128, 6, 2, O), F16)  # rows: pre|hidden|side
    gbs_d = din("gb_side", (SH, 4), F32)
    gbc_d = din("gb_cat", (O, 4), F32)
    ident_d = din("ident", (128, 128), F16)
    mask_d = din("mask", (SH, S), F16)

    score_d = nc.dram_tensor("score", [R, S, V], F16,
                             kind="ExternalOutput").ap()

    with tile.TileContext(nc) as tc:
        with tc.tile_pool(name="const", bufs=1) as const_p, \
             tc.tile_pool(name="big", bufs=1) as big_p, \
             tc.tile_pool(name="mring", bufs=10) as m_p, \
             tc.tile_pool(name="small", bufs=1) as sm_p, \
             tc.tile_pool(name="scoresb", bufs=3) as sc_p, \
             tc.tile_pool(name="psmm", bufs=4, space="PSUM") as psmm, \
             tc.tile_pool(name="pssc", bufs=3, space="PSUM") as pssc, \
             tc.tile_pool(name="dram", bufs=1, space="DRAM") as dram_p:

            # ============ constant/small loads (SP queue) ============
            ident = const_p.tile([128, 128], F16)
            nc.sync.dma_start(ident[:], ident_d)
            eps_t = const_p.tile([128, 1], F32)
            nc.vector.memset(eps_t[:], EPS)
            sqrt_warm = const_p.tile([128, 1], F32, name="sqrt_warm")
            nc.scalar.activation(sqrt_warm[:], eps_t[:], AF.Sqrt,
                                 bias=eps_t[:])
            sfT_sb = const_p.tile([SF, 2, S], F16)
            nc.sync.dma_start(sfT_sb[:], sfT_d)
            wside_sb = const_p.tile([SF, 2, SH], F16)
            nc.sync.dma_start(wside_sb[:], wside_d)
            q_sb = const_p.tile([O, R, O], F16)
            nc.sync.dma_start(q_sb[:], q_d)
            wcat_sb = const_p.tile([128, 6, 2, O], F16)
            nc.sync.dma_start(wcat_sb[:], wcat_d)
            gbs_sb = const_p.tile([SH, 4], F32)
            nc.sync.dma_start(gbs_sb[:], gbs_d)
            gbc_sb = const_p.tile([O, 4], F32)
            nc.sync.dma_start(gbc_sb[:], gbc_d)
            mask_sb = const_p.tile([SH, S], F16)
            nc.sync.dma_start(mask_sb[:], mask_d)

            # ============ big stream (ACT queue, exact order) ============
            w_sb = big_p.tile([128, KT, RH], F16)
            nc.scalar.dma_start(w_sb[:], w_d)
            fvT_sb = big_p.tile([128, KT, S], F16)
            nc.scalar.dma_start(fvT_sb[:, :, 0:128], fvT_d[:, :, 0:128])
            nc.scalar.dma_start(fvT_sb[:, :, 128:S], fvT_d[:, :, 128:S])
            fuT_sb = big_p.tile([128, KT, S], F16)
            nc.scalar.dma_start(fuT_sb[:], fuT_d)
            muT_sb = [m_p.tile([128, 2, UP], FP8, tag="mT", name=f"muT_{r}")
                      for r in range(R)]
            mvT_sb = [m_p.tile([128, 2, UP], FP8, tag="mT", name=f"mvT_{r}")
                      for r in range(R)]
            for r in range(R):
                nc.scalar.dma_start(muT_sb[r][:], muT_d[r])
            for r in range(R):
                nc.scalar.dma_start(mvT_sb[r][:], mvT_d[r])

            # ============ collective buffers ============
            replica = [list(range(NC))]
            rs_in = dram_p.tile([NC, 2, H, R, SP2], F16)
            rs_out = dram_p.tile([2, H, R, SP2], F16)
            ag_in = dram_p.tile([O, S + 4], F32)
            ag_out = dram_p.tile([NC, O, S + 4], F32, addr_space="Shared")

            # catT: [128, 6, S] f16 per side; rows pre(0:320)|hidden|side
            catT = [big_p.tile([128, 6, S], F16, name=f"catT{sd}")
                    for sd in range(2)]
            for sd in range(2):
                nc.vector.memset(catT[sd][SH:128, 5, :], 0.0)

            # partial-hidden staging, shared by both sides via WAR reuse
            stage = big_p.tile([H, NC, R, SP2], F16, name="stage")
            nc.vector.memset(stage[:, :, :, S:SP2], 0.0)

            # ============ side matmuls + BN partial sums ============
            s_loc = sm_p.tile([SH, 2, S], F32)
            junk = sm_p.tile([128, S], F32, name="junk")

            s_sums = [sm_p.tile([SH, 2], F32, name=f"s_sums{sd}")
                      for sd in range(2)]

            def side_compute(sd):
                ps_s = psmm.tile([SH, S], F32, tag="mm", name="ps_side")
                nc.tensor.matmul(ps_s[:], wside_sb[:, sd, :], sfT_sb[:, sd, :],
                                 start=True, stop=True)
                nc.vector.tensor_copy(s_loc[:, sd, :], ps_s[:])
                nc.vector.reduce_sum(s_sums[sd][:, 0:1], s_loc[:, sd, :],
                                     axis=AXX)
                nc.vector.tensor_mul(junk[0:SH, :], s_loc[:, sd, :],
                                     s_loc[:, sd, :])
                nc.vector.reduce_sum(s_sums[sd][:, 1:2], junk[0:SH, :],
                                     axis=AXX)

            def side_stage(sd):
                # replicate into every dest shard of the RS payload (row r=0)
                for c in range(NC):
                    nc.vector.tensor_copy(stage[:, c, 0, S:S + 2],
                                          s_sums[sd][:])

            # ============ projection: pre[row, rh] = f^T W ============
            prevh = [big_p.tile([128, 2, RH], F16, name=f"prevh_{sd}")
                     for sd in range(2)]
            prevh8 = [big_p.tile([128, 2, RH], FP8, name=f"prevh8_{sd}")
                      for sd in range(2)]

            def proj_side(sd, fT):
                pre_f16 = prevh[sd]
                for ch in range(2):
                    ps_pre = psmm.tile([128, RH], F32, tag="mm", name="ps_pre")
                    for k in range(KT):
                        nc.tensor.matmul(ps_pre[:],
                                         fT[:, k, ch * 128:(ch + 1) * 128],
                                         w_sb[:, k, :],
                                         start=(k == 0), stop=(k == KT - 1))
                    nc.vector.tensor_copy(pre_f16[:, ch, :], ps_pre[:])
                    nc.scalar.activation(prevh8[sd][:, ch, :], ps_pre[:],
                                         AF.Copy, scale=P_SC)

            # deferred: transpose prevh into catT pre rows during the RS
            # window (keeps these copies off the pre-RS DVE critical path)
            def catT_pre_fill(sd):
                for ch in range(2):
                    for c in range(3):
                        cw = min(128, RH - c * 128)
                        ps_t = psmm.tile([128, 128], F16, tag="mm",
                                         name="ps_t")
                        nc.tensor.transpose(
                            ps_t[0:cw, :],
                            prevh[sd][:, ch, c * 128:c * 128 + cw],
                            ident[:])
                        row = c * 128
                        blk, off = divmod(row, 128)
                        eng = nc.vector if (ch + c) % 2 == 0 else None
                        if eng is None:
                            nc.scalar.copy(
                                catT[sd][off:off + cw, blk,
                                         ch * 128:(ch + 1) * 128],
                                ps_t[0:cw, :])
                        else:
                            nc.vector.tensor_copy(
                                catT[sd][off:off + cw, blk,
                                         ch * 128:(ch + 1) * 128],
                                ps_t[0:cw, :])

            # ==== partial hidden (fp8 DoubleRow) -> f16 stage -> DMA ====
            def partial_side(sd, mT, osd):
                for r in range(R):
                    pss = [psmm.tile([H, 2, S], F32, tag="mm", name=f"ps_p{g}")
                           for g in range(4)]
                    for g in range(4):
                        for c in range(2):
                            nc.tensor.matmul(
                                pss[g][:, c, :],
                                prevh8[osd][:, :, r * H:(r + 1) * H],
                                mT[r][:, :, (2 * g + c) * S:(2 * g + c + 1) * S],
                                start=True, stop=True, perf_mode=DR)
                    for g in range(4):
                        dst = stage[:, 2 * g:2 * g + 2, r, 0:S]
                        if g % 2 == 0:
                            nc.vector.tensor_copy(dst, pss[g][:])
                        else:
                            nc.scalar.copy(dst, pss[g][:])
                    nc.sync.dma_start(
                        rs_in[:, sd, :, r, :].rearrange("c h x -> h c x"),
                        stage[:, :, r, :])

            side_compute(0)
            side_compute(1)
            proj_side(1, fvT_sb)
            proj_side(0, fuT_sb)
            side_stage(0)
            partial_side(0, muT_sb, 1)
            side_stage(1)
            partial_side(1, mvT_sb, 0)
            catT_pre_fill(1)
            catT_pre_fill(0)
            nc.gpsimd.collective_compute("ReduceScatter", ALU.add,
                                         replica_groups=replica,
                                         ins=[rs_in.opt()],
                                         outs=[rs_out.opt()])



            # ============ BN helpers (both sides batched: [P, 2]) ======
            def bn_from_sums(tg, sums, sumsq, g_col, b_col, n, P):
                def t(nm):
                    return sm_p.tile([P, 2], F32, tag=f"{nm}_{tg}",
                                     name=f"{nm}_{tg}")
                mu = t("bn_mu")
                nc.vector.tensor_scalar_mul(mu[:], sums[:], 1.0 / n)
                e2 = t("bn_e2")
                nc.vector.tensor_scalar_mul(e2[:], sumsq[:], 1.0 / n)
                var = t("bn_var")
                nc.vector.tensor_mul(var[:], mu[:], mu[:])
                nc.vector.tensor_sub(var[:], e2[:], var[:])
                std = t("bn_std")
                nc.scalar.activation(std[:], var[:], AF.Sqrt, bias=eps_t[0:P, :])
                rstd = t("bn_rstd")
                nc.vector.reciprocal(rstd[:], std[:])
                scale = t("bn_scale")
                nc.vector.tensor_mul(scale[:], g_col, rstd[:])
                shift = t("bn_shift")
                nc.vector.tensor_mul(shift[:], mu[:], scale[:])
                nc.vector.tensor_sub(shift[:], b_col, shift[:])
                return scale, shift

            # ====== hidden relu into catT rows 320:640; side BN ======
            side_tmp = sm_p.tile([SH, 2, S], F16, name="side_tmp")
            hsum = sm_p.tile([H, 2, R, SP2], F16, name="hsum")
            nc.sync.dma_start(hsum[:], rs_out.rearrange("s h r x -> h s r x"))
            t_sums = sm_p.tile([SH, 2, 2], F32, name="t_sums")
            nc.vector.tensor_copy(t_sums[:], hsum[:, :, 0, S:S + 2])
            for sd in range(2):
                for r in range(R):
                    row = RH + r * H
                    blk, off = divmod(row, 128)
                    if r % 2 == 0:
                        nc.scalar.activation(catT[sd][off:off + H, blk, :],
                                             hsum[:, sd, r, 0:S],
                                             AF.Relu, scale=HID_SC)
                    else:
                        nc.vector.tensor_scalar(
                            catT[sd][off:off + H, blk, :], hsum[:, sd, r, 0:S],
                            HID_SC, 0.0, op0=ALU.mult, op1=ALU.max)
            sc2, sh2 = bn_from_sums("sB", t_sums[:, :, 0], t_sums[:, :, 1],
                                    gbs_sb[:, 0:2], gbs_sb[:, 2:4], U, SH)
            for sd in range(2):
                nc.scalar.activation(side_tmp[:, sd, :], s_loc[:, sd, :],
                                     AF.Relu, bias=sh2[:, sd:sd + 1],
                                     scale=sc2[:, sd:sd + 1])
                nc.vector.tensor_mul(catT[sd][0:SH, 5, :], side_tmp[:, sd, :],
                                     mask_sb[:])

            # ============ cat matmul (f16) + slim AG ============
            y_sb = [sm_p.tile([O, S], F32, name=f"y_sb{sd}") for sd in range(2)]
            stats = sm_p.tile([O, 4], F32, name="stats")

            for sd in range(2):
                ps_y = psmm.tile([O, S], F32, tag="mm", name="ps_y")
                for b in range(6):
                    nc.tensor.matmul(ps_y[:], wcat_sb[:, b, sd, :],
                                     catT[sd][:, b, :],
                                     start=(b == 0), stop=(b == 5))
                nc.vector.tensor_copy(y_sb[sd][:], ps_y[:])
                nc.vector.reduce_sum(stats[:, sd:sd + 1], y_sb[sd][:],
                                     axis=AXX)
                nc.vector.tensor_mul(junk[0:O, :], y_sb[sd][:],
                                     y_sb[sd][:])
                nc.vector.reduce_sum(stats[:, 2 + sd:3 + sd],
                                     junk[0:O, :], axis=AXX)
            ag_st = sm_p.tile([O, S + 4], F32, name="ag_st")
            nc.vector.tensor_copy(ag_st[:, 0:S], y_sb[1][:])
            nc.vector.tensor_copy(ag_st[:, S:S + 4], stats[:])
            nc.sync.dma_start(ag_in[:], ag_st[:])
            nc.gpsimd.collective_compute("AllGather", ALU.bypass,
                                         replica_groups=replica,
                                         ins=[ag_in.opt()],
                                         outs=[ag_out.opt()])

            yv_all = sm_p.tile([O, NC, S + 4], F32, name="yv_all")
            nc.sync.dma_start(yv_all[:, :, S:S + 4],
                              ag_out[:, :, S:S + 4].rearrange("c p x -> p c x"))
            nc.sync.dma_start(yv_all[:, 0:4, 0:S],
                              ag_out[0:4, :, 0:S].rearrange("c p x -> p c x"))
            nc.sync.dma_start(yv_all[:, 4:8, 0:S],
                              ag_out[4:8, :, 0:S].rearrange("c p x -> p c x"))

            # ============ cat BN (global stats) + embeds ============
            statacc = sm_p.tile([O, 4], F32, name="statacc")
            nc.vector.tensor_copy(statacc[:], yv_all[:, 0, S:S + 4])
            for c in range(1, NC):
                nc.vector.tensor_add(statacc[:], statacc[:],
                                     yv_all[:, c, S:S + 4])
            embed_u = sm_p.tile([O, S], F16)
            embed_v = sm_p.tile([O, UP], F16)
            scc, shc = bn_from_sums("cB", statacc[:, 0:2], statacc[:, 2:4],
                                    gbc_sb[:, 0:2], gbc_sb[:, 2:4], U, O)
            nc.scalar.activation(embed_u[:], y_sb[0][:], AF.Relu,
                                 bias=shc[:, 0:1], scale=scc[:, 0:1])
            ev = embed_v.rearrange("p (c u) -> p c u", c=NC)
            for hf in range(2):
                nc.scalar.activation(ev[:, 4 * hf:4 * hf + 4, :],
                                     yv_all[:, 4 * hf:4 * hf + 4, 0:S],
                                     AF.Relu, bias=shc[:, 1:2],
                                     scale=scc[:, 1:2])

            # ============ score ============
            for r in range(R):
                ps_t1 = psmm.tile([O, S], F32, tag="mm", name="ps_t1")
                nc.tensor.matmul(ps_t1[:], q_sb[:, r, :], embed_u[:],
                                 start=True, stop=True)
                t1 = sc_p.tile([O, S], F16, tag="t1", name="t1")
                nc.vector.tensor_copy(t1[:], ps_t1[:])
                for ch in range(2):
                    out_sb = sc_p.tile([128, V], F16, tag="osb", name="out_sb")
                    for i, (n0, nn) in enumerate(NTILES):
                        ps_sc = pssc.tile([128, 512], F32, tag="sc",
                                          name="ps_sc")
                        nc.tensor.matmul(ps_sc[:, 0:nn],
                                         t1[:, ch * 128:(ch + 1) * 128],
                                         embed_v[:, n0:n0 + nn],
                                         start=True, stop=True)
                        if i % 2 == 0:
                            nc.vector.tensor_copy(out_sb[:, n0:n0 + nn],
                                                  ps_sc[:, 0:nn])
                        else:
                            nc.scalar.copy(out_sb[:, n0:n0 + nn],
                                           ps_sc[:, 0:nn])
                    nc.sync.dma_start(score_d[r, ch * 128:(ch + 1) * 128, :],
                                      out_sb[:])

    nc.compile()
    return nc


def _prep(inputs):
    """Host-side shard/pad/scale/cast/pack. Returns in_maps for 8 cores."""
    def padto(a, n, axis):
        pad = [(0, 0)] * a.ndim
        pad[axis] = (0, n - a.shape[axis])
        return np.pad(a, pad)

    f32 = np.float32
    fu = padto(padto(np.asarray(inputs['feature_u'], f32), UP, 0), UP, 1)
    fv = padto(padto(np.asarray(inputs['feature_v'], f32), UP, 0), UP, 1)
    Mu = padto(padto(np.asarray(inputs['M_u'], f32), UP, 1), UP, 2) * M_SC
    Mv = padto(padto(np.asarray(inputs['M_v'], f32), UP, 1), UP, 2) * M_SC
    W = padto(np.asarray(inputs['W'], f32), UP, 1)
    sfu = padto(np.asarray(inputs['side_feature_u'], f32), UP, 0)
    sfv = padto(np.asarray(inputs['side_feature_v'], f32), UP, 0)

    # catT row order [pre | hidden | side]; reference cat order is
    # [hidden | f@W | side] -> permute w_cat rows to match.
    perm = np.concatenate([np.arange(RH, 2 * RH), np.arange(0, RH),
                           np.arange(2 * RH, 2 * RH + SH)])
    wcat = np.stack(
        [padto(np.asarray(inputs[f'w_cat_{s}'], f32)[perm], 6 * 128, 0)
         for s in ('u', 'v')], 1)                   # [768, 2, 75]
    wcat16 = np.ascontiguousarray(
        wcat.reshape(6, 128, 2, O).transpose(1, 0, 2, 3)).astype(F16N)
    wside = np.stack([np.asarray(inputs['w_side_u'], f32),
                      np.asarray(inputs['w_side_v'], f32)], 1).astype(F16N)
    gbs = np.stack([inputs['g_side_u'], inputs['g_side_v'],
                    inputs['beta_side_u'], inputs['beta_side_v']],
                   1).astype(f32)
    gbc = np.stack([inputs['g_cat_u'], inputs['g_cat_v'],
                    inputs['beta_cat_u'], inputs['beta_cat_v']],
                   1).astype(f32)
    # W repacked to [p, k, r*h] so each k-slice is a contiguous [128, RH] rhs
    w16 = np.ascontiguousarray(
        W.reshape(R, KT, 128, H).transpose(2, 1, 0, 3)).reshape(
        128, KT, RH).astype(F16N)
    q16 = np.ascontiguousarray(
        np.asarray(inputs['Q'], f32).transpose(1, 0, 2)).astype(F16N)

    def pack_f(feat, sl):
        # [2048, 256] rows sl -> [128, 16, 256]: f = k*128 + p
        a = np.ascontiguousarray(feat[sl].T)        # [2048 f, 256]
        return np.ascontiguousarray(
            a.reshape(KT, 128, S).transpose(1, 0, 2)).astype(F16N)

    def pack_m(Msc, r, sl):
        # M[r][:, my rows].T -> [128, 2, 2048]: local row = t*128 + p
        a = np.ascontiguousarray(Msc[r][:, sl].T)   # [256 local, 2048]
        return np.ascontiguousarray(
            a.reshape(2, 128, UP).transpose(1, 0, 2)).astype(F8N)

    in_maps = []
    for c in range(NC):
        sl = slice(c * S, (c + 1) * S)
        in_maps.append({
            "fuT": pack_f(fu, sl),
            "fvT": pack_f(fv, sl),
            "muT": np.stack([pack_m(Mu, r, sl) for r in range(R)]),
            "mvT": np.stack([pack_m(Mv, r, sl) for r in range(R)]),
            "w": w16,
            "q": q16,
            "sfT": np.ascontiguousarray(
                np.stack([sfu[sl].T, sfv[sl].T], 1)).astype(F16N),
            "wside": wside,
            "wcat": wcat16,
            "gb_side": gbs,
            "gb_cat": gbc,
            "ident": np.eye(128, dtype=F16N),
            "mask": np.broadcast_to(
                (np.arange(c * S, (c + 1) * S) < U).astype(F16N),
                (SH, S)).copy(),
        })
    return in_maps


def kernel(**inputs) -> np.ndarray:
    if "nc" not in _CACHE:
        _CACHE["nc"] = _build()
    nc = _CACHE["nc"]
    in_maps = _prep(inputs)
    res = bass_utils.run_bass_kernel_spmd(nc, in_maps, core_ids=list(range(NC)))
    score = np.concatenate(
        [np.asarray(res.results[c]["score"]) for c in range(NC)],
        axis=1).astype(np.float32)
    return score[:, :U, :]


if __name__ == "__main__":
    print("kernel module OK")


# revision 50
# speedup vs baseline: 1.0538x; 1.0280x over previous
"""Trainium2 Bass kernel for nn_GCMC (GNN message passing / GCMC scoring).

v5 strategy: row-shard users AND items across 8 NeuronCores (256 padded rows
each). Message passing is ONE merged ReduceScatter: each core column-shards
M (its 256 v-columns of M_u, u-columns of M_v), computes partial hidden sums
for ALL opposite-side rows from its local projection slice, and a single
ReduceScatter (add) over a [NC, 2, H, R, 258] fp16 payload returns both
sides' per-core hidden rows (side-branch BatchNorm partial sums ride in 2
extra columns). One slim AllGather then shares the pre-BN v-side cat output
y_v plus both sides' cat BatchNorm partial sums, so every core computes
global BatchNorm stats locally and the full embed_v for the final bilinear
score.

Precision: fp16 operands (the PE accumulates in fp32; fp16 measured no worse
than bf16 here), fp32 catT is not needed (fp16 catT/w_cat measured fine), M
and the projection copy it contracts with travel as fp8e4m3 (x64 / x4
scales, undone by the hidden-relu activation scale 2^-8) enabling DoubleRow
matmuls and halving the dominant M DMA traffic. The RS payload is fp16; the
AllGather payload MUST stay fp32 - 16-bit AllGather payloads measurably
degrade (~1.5% extra score error, consistent with the collective
round-tripping 16-bit data through bf16), while the fp16 ReduceScatter only
costs ~0.35%. Measured end-to-end max-rel error: 1.41% vs the 2% gate.

Collectives: 1x fp16 ReduceScatter (23.3us) + 1x fp32 AllGather (30.6us) on
the serial collective device, vs 2 RS + 1 AG = 77us in v2.1. A dummy Sqrt
activation at t=0 preloads the activation-function table so the BatchNorm
Sqrt does not pay a 1.3us table load on the post-RS critical path.
"""
import sys
if '/opt/trn_rl_repo' not in sys.path:
    sys.path.insert(0, '/opt/trn_rl_repo')

import numpy as np
import ml_dtypes

import concourse.bass as bass
import concourse.bacc as bacc
import concourse.mybir as mybir
import concourse.tile as tile
from concourse import bass_utils

F16N = np.float16
F8N = ml_dtypes.float8_e4m3
F32 = mybir.dt.float32
F16 = mybir.dt.float16
FP8 = mybir.dt.float8e4
AF = mybir.ActivationFunctionType
ALU = mybir.AluOpType
AXX = mybir.AxisListType.X
DR = mybir.MatmulPerfMode.DoubleRow

U = V = F = 2000
R, H, O, SH, SF = 5, 64, 75, 64, 128
RH = R * H           # 320
UP = 2048            # padded U/V/F
S = 256              # rows per core
SP2 = S + 2          # RS payload row width (256 data + 2 BN-sum cols)
NC = 8
KT = 16              # 128-row k-tiles over the padded 2048 contraction dims
EPS = 1e-5
M_SC, P_SC = 64.0, 4.0          # fp8 scales for M and prevh
HID_SC = 1.0 / (M_SC * P_SC)    # 2^-8, folded into hidden relu
NTILES = [(0, 512), (512, 512), (1024, 512), (1536, 464)]  # score v-tiles

_CACHE = {}


def _build():
    nc = bacc.Bacc("TRN2", target_bir_lowering=False, debug=False,
                   num_devices=NC)

    def din(name, shape, dt):
        return nc.dram_tensor(name, list(shape), dt, kind="ExternalInput").ap()

    fuT_d = din("fuT", (128, KT, S), F16)     # my u rows, [f, kt, u]
    fvT_d = din("fvT", (128, KT, S), F16)
    muT_d = din("muT", (R, 128, 2, UP), FP8)  # x64 M_u[r][:, my_v].T packed
    mvT_d = din("mvT", (R, 128, 2, UP), FP8)
    w_d = din("w", (128, KT, RH), F16)
    q_d = din("q", (O, R, O), F16)
    sfT_d = din("sfT", (SF, 2, S), F16)
    wside_d = din("wside", (SF, 2, SH), F16)
    wcat_d = din("wcat", (128, 6, 2, O), F16)  # rows: pre|hidden|side
    gbs_d = din("gb_side", (SH, 4), F32)
    gbc_d = din("gb_cat", (O, 4), F32)
    ident_d = din("ident", (128, 128), F16)
    mask_d = din("mask", (SH, S), F16)

    score_d = nc.dram_tensor("score", [R, S, V], F16,
                             kind="ExternalOutput").ap()

    with tile.TileContext(nc) as tc:
        with tc.tile_pool(name="const", bufs=1) as const_p, \
             tc.tile_pool(name="big", bufs=1) as big_p, \
             tc.tile_pool(name="mring", bufs=10) as m_p, \
             tc.tile_pool(name="small", bufs=1) as sm_p, \
             tc.tile_pool(name="scoresb", bufs=3) as sc_p, \
             tc.tile_pool(name="psmm", bufs=4, space="PSUM") as psmm, \
             tc.tile_pool(name="pssc", bufs=3, space="PSUM") as pssc, \
             tc.tile_pool(name="dram", bufs=1, space="DRAM") as dram_p:

            # ============ constant/small loads (SP queue) ============
            ident = const_p.tile([128, 128], F16)
            nc.sync.dma_start(ident[:], ident_d)
            eps_t = const_p.tile([128, 1], F32)
            nc.vector.memset(eps_t[:], EPS)
            sqrt_warm = const_p.tile([128, 1], F32, name="sqrt_warm")
            nc.scalar.activation(sqrt_warm[:], eps_t[:], AF.Sqrt,
                                 bias=eps_t[:])
            sfT_sb = const_p.tile([SF, 2, S], F16)
            nc.sync.dma_start(sfT_sb[:], sfT_d)
            wside_sb = const_p.tile([SF, 2, SH], F16)
            nc.sync.dma_start(wside_sb[:], wside_d)
            q_sb = const_p.tile([O, R, O], F16)
            nc.sync.dma_start(q_sb[:], q_d)
            wcat_sb = const_p.tile([128, 6, 2, O], F16)
            nc.sync.dma_start(wcat_sb[:], wcat_d)
            gbs_sb = const_p.tile([SH, 4], F32)
            nc.sync.dma_start(gbs_sb[:], gbs_d)
            gbc_sb = const_p.tile([O, 4], F32)
            nc.sync.dma_start(gbc_sb[:], gbc_d)
            mask_sb = const_p.tile([SH, S], F16)
            nc.sync.dma_start(mask_sb[:], mask_d)

            # ============ big stream (ACT queue, exact order) ============
            w_sb = big_p.tile([128, KT, RH], F16)
            nc.scalar.dma_start(w_sb[:], w_d)
            fvT_sb = big_p.tile([128, KT, S], F16)
            nc.scalar.dma_start(fvT_sb[:, :, 0:128], fvT_d[:, :, 0:128])
            nc.scalar.dma_start(fvT_sb[:, :, 128:S], fvT_d[:, :, 128:S])
            fuT_sb = big_p.tile([128, KT, S], F16)
            nc.scalar.dma_start(fuT_sb[:], fuT_d)
            muT_sb = [m_p.tile([128, 2, UP], FP8, tag="mT", name=f"muT_{r}")
                      for r in range(R)]
            mvT_sb = [m_p.tile([128, 2, UP], FP8, tag="mT", name=f"mvT_{r}")
                      for r in range(R)]
            for r in range(R):
                nc.scalar.dma_start(muT_sb[r][:], muT_d[r])
            for r in range(R):
                nc.scalar.dma_start(mvT_sb[r][:], mvT_d[r])

            # ============ collective buffers ============
            replica = [list(range(NC))]
            rs_in = dram_p.tile([NC, 2, H, R, SP2], F16)
            rs_out = dram_p.tile([2, H, R, SP2], F16)
            ag_in = dram_p.tile([O, S + 4], F32)
            ag_out = dram_p.tile([NC, O, S + 4], F32, addr_space="Shared")

            # catT: [128, 6, S] f16 per side; rows pre(0:320)|hidden|side
            catT = [big_p.tile([128, 6, S], F16, name=f"catT{sd}")
                    for sd in range(2)]
            for sd in range(2):
                nc.vector.memset(catT[sd][SH:128, 5, :], 0.0)

            # partial-hidden staging, one tile per side (avoids WAR
            # serialization of v-side copies behind u-side stage DMAs)
            stages = [big_p.tile([H, NC, R, SP2], F16, name=f"stage{sd}")
                      for sd in range(2)]
            for sd in range(2):
                nc.vector.memset(stages[sd][:, :, :, S:SP2], 0.0)

            # ============ side matmuls + BN partial sums ============
            s_loc = sm_p.tile([SH, 2, S], F32)
            junk = sm_p.tile([128, S], F32, name="junk")

            s_sums = [sm_p.tile([SH, 2], F32, name=f"s_sums{sd}")
                      for sd in range(2)]

            def side_compute(sd):
                ps_s = psmm.tile([SH, S], F32, tag="mm", name="ps_side")
                nc.tensor.matmul(ps_s[:], wside_sb[:, sd, :], sfT_sb[:, sd, :],
                                 start=True, stop=True)
                nc.vector.tensor_copy(s_loc[:, sd, :], ps_s[:])
                nc.vector.reduce_sum(s_sums[sd][:, 0:1], s_loc[:, sd, :],
                                     axis=AXX)
                nc.vector.tensor_mul(junk[0:SH, :], s_loc[:, sd, :],
                                     s_loc[:, sd, :])
                nc.vector.reduce_sum(s_sums[sd][:, 1:2], junk[0:SH, :],
                                     axis=AXX)

            def side_stage(sd):
                # replicate into every dest shard of the RS payload (row r=0)
                for c in range(NC):
                    nc.vector.tensor_copy(stages[sd][:, c, 0, S:S + 2],
                                          s_sums[sd][:])

            # ============ projection: pre[row, rh] = f^T W ============
            prevh = [big_p.tile([128, 2, RH], F16, name=f"prevh_{sd}")
                     for sd in range(2)]
            prevh8 = [big_p.tile([128, 2, RH], FP8, name=f"prevh8_{sd}")
                      for sd in range(2)]

            def proj_side(sd, fT):
                pre_f16 = prevh[sd]
                for ch in range(2):
                    ps_pre = psmm.tile([128, RH], F32, tag="mm", name="ps_pre")
                    for k in range(KT):
                        nc.tensor.matmul(ps_pre[:],
                                         fT[:, k, ch * 128:(ch + 1) * 128],
                                         w_sb[:, k, :],
                                         start=(k == 0), stop=(k == KT - 1))
                    nc.vector.tensor_copy(pre_f16[:, ch, :], ps_pre[:])
                    nc.scalar.activation(prevh8[sd][:, ch, :], ps_pre[:],
                                         AF.Copy, scale=P_SC)

            # deferred: transpose prevh into catT pre rows during the RS
            # window (keeps these copies off the pre-RS DVE critical path)
            def catT_pre_fill(sd):
                for ch in range(2):
                    for c in range(3):
                        cw = min(128, RH - c * 128)
                        ps_t = psmm.tile([128, 128], F16, tag="mm",
                                         name="ps_t")
                        nc.tensor.transpose(
                            ps_t[0:cw, :],
                            prevh[sd][:, ch, c * 128:c * 128 + cw],
                            ident[:])
                        row = c * 128
                        blk, off = divmod(row, 128)
                        eng = nc.vector if (ch + c) % 2 == 0 else None
                        if eng is None:
                            nc.scalar.copy(
                                catT[sd][off:off + cw, blk,
                                         ch * 128:(ch + 1) * 128],
                                ps_t[0:cw, :])
                        else:
                            nc.vector.tensor_copy(
                                catT[sd][off:off + cw, blk,
                                         ch * 128:(ch + 1) * 128],
                                ps_t[0:cw, :])

            # ==== partial hidden (fp8 DoubleRow) -> f16 stage -> DMA ====
            def partial_side(sd, mT, osd):
                stage = stages[sd]
                for r in range(R):
                    pss = [psmm.tile([H, 2, S], F32, tag="mm", name=f"ps_p{g}")
                           for g in range(4)]
                    for g in range(4):
                        for c in range(2):
                            nc.tensor.matmul(
                                pss[g][:, c, :],
                                prevh8[osd][:, :, r * H:(r + 1) * H],
                                mT[r][:, :, (2 * g + c) * S:(2 * g + c + 1) * S],
                                start=True, stop=True, perf_mode=DR)
                    for g in range(4):
                        dst = stage[:, 2 * g:2 * g + 2, r, 0:S]
                        if g % 2 == 0:
                            nc.vector.tensor_copy(dst, pss[g][:])
                        else:
                            nc.scalar.copy(dst, pss[g][:])
                    nc.sync.dma_start(
                        rs_in[:, sd, :, r, :].rearrange("c h x -> h c x"),
                        stage[:, :, r, :])

            side_compute(0)
            side_compute(1)
            proj_side(1, fvT_sb)
            proj_side(0, fuT_sb)
            side_stage(0)
            partial_side(0, muT_sb, 1)
            side_stage(1)
            partial_side(1, mvT_sb, 0)
            catT_pre_fill(1)
            catT_pre_fill(0)
            nc.gpsimd.collective_compute("ReduceScatter", ALU.add,
                                         replica_groups=replica,
                                         ins=[rs_in.opt()],
                                         outs=[rs_out.opt()])



            # ============ BN helpers (both sides batched: [P, 2]) ======
            def bn_from_sums(tg, sums, sumsq, g_col, b_col, n, P):
                def t(nm):
                    return sm_p.tile([P, 2], F32, tag=f"{nm}_{tg}",
                                     name=f"{nm}_{tg}")
                mu = t("bn_mu")
                nc.vector.tensor_scalar_mul(mu[:], sums[:], 1.0 / n)
                e2 = t("bn_e2")
                nc.vector.tensor_scalar_mul(e2[:], sumsq[:], 1.0 / n)
                var = t("bn_var")
                nc.vector.tensor_mul(var[:], mu[:], mu[:])
                nc.vector.tensor_sub(var[:], e2[:], var[:])
                std = t("bn_std")
                nc.scalar.activation(std[:], var[:], AF.Sqrt, bias=eps_t[0:P, :])
                rstd = t("bn_rstd")
                nc.vector.reciprocal(rstd[:], std[:])
                scale = t("bn_scale")
                nc.vector.tensor_mul(scale[:], g_col, rstd[:])
                shift = t("bn_shift")
                nc.vector.tensor_mul(shift[:], mu[:], scale[:])
                nc.vector.tensor_sub(shift[:], b_col, shift[:])
                return scale, shift

            # ====== hidden relu into catT rows 320:640; side BN ======
            side_tmp = sm_p.tile([SH, 2, S], F16, name="side_tmp")
            hsum = sm_p.tile([H, 2, R, SP2], F16, name="hsum")
            nc.sync.dma_start(hsum[:], rs_out.rearrange("s h r x -> h s r x"))
            t_sums = sm_p.tile([SH, 2, 2], F32, name="t_sums")
            nc.vector.tensor_copy(t_sums[:], hsum[:, :, 0, S:S + 2])
            for sd in range(2):
                for r in range(R):
                    row = RH + r * H
                    blk, off = divmod(row, 128)
                    if r % 2 == 0:
                        nc.scalar.activation(catT[sd][off:off + H, blk, :],
                                             hsum[:, sd, r, 0:S],
                                             AF.Relu, scale=HID_SC)
                    else:
                        nc.vector.tensor_scalar(
                            catT[sd][off:off + H, blk, :], hsum[:, sd, r, 0:S],
                            HID_SC, 0.0, op0=ALU.mult, op1=ALU.max)
            sc2, sh2 = bn_from_sums("sB", t_sums[:, :, 0], t_sums[:, :, 1],
                                    gbs_sb[:, 0:2], gbs_sb[:, 2:4], U, SH)
            for sd in range(2):
                nc.scalar.activation(side_tmp[:, sd, :], s_loc[:, sd, :],
                                     AF.Relu, bias=sh2[:, sd:sd + 1],
                                     scale=sc2[:, sd:sd + 1])
                nc.vector.tensor_mul(catT[sd][0:SH, 5, :], side_tmp[:, sd, :],
                                     mask_sb[:])

            # ============ cat matmul (f16) + slim AG ============
            y_sb = [sm_p.tile([O, S], F32, name=f"y_sb{sd}") for sd in range(2)]
            stats = sm_p.tile([O, 4], F32, name="stats")

            for sd in range(2):
                ps_y = psmm.tile([O, S], F32, tag="mm", name="ps_y")
                for b in range(6):
                    nc.tensor.matmul(ps_y[:], wcat_sb[:, b, sd, :],
                                     catT[sd][:, b, :],
                                     start=(b == 0), stop=(b == 5))
                nc.vector.tensor_copy(y_sb[sd][:], ps_y[:])
                nc.vector.reduce_sum(stats[:, sd:sd + 1], y_sb[sd][:],
                                     axis=AXX)
                nc.vector.tensor_mul(junk[0:O, :], y_sb[sd][:],
                                     y_sb[sd][:])
                nc.vector.reduce_sum(stats[:, 2 + sd:3 + sd],
                                     junk[0:O, :], axis=AXX)
            ag_st = sm_p.tile([O, S + 4], F32, name="ag_st")
            nc.vector.tensor_copy(ag_st[:, 0:S], y_sb[1][:])
            nc.vector.tensor_copy(ag_st[:, S:S + 4], stats[:])
            nc.sync.dma_start(ag_in[:], ag_st[:])
            nc.gpsimd.collective_compute("AllGather", ALU.bypass,
                                         replica_groups=replica,
                                         ins=[ag_in.opt()],
                                         outs=[ag_out.opt()])

            yv_all = sm_p.tile([O, NC, S + 4], F32, name="yv_all")
            nc.sync.dma_start(yv_all[:, :, S:S + 4],
                              ag_out[:, :, S:S + 4].rearrange("c p x -> p c x"))
            nc.sync.dma_start(yv_all[:, 0:4, 0:S],
                              ag_out[0:4, :, 0:S].rearrange("c p x -> p c x"))
            nc.sync.dma_start(yv_all[:, 4:8, 0:S],
                              ag_out[4:8, :, 0:S].rearrange("c p x -> p c x"))

            # ============ cat BN (global stats) + embeds ============
            statacc = sm_p.tile([O, 4], F32, name="statacc")
            nc.vector.tensor_copy(statacc[:], yv_all[:, 0, S:S + 4])
            for c in range(1, NC):
                nc.vector.tensor_add(statacc[:], statacc[:],
                                     yv_all[:, c, S:S + 4])
            embed_u = sm_p.tile([O, S], F16)
            embed_v = sm_p.tile([O, UP], F16)
            scc, shc = bn_from_sums("cB", statacc[:, 0:2], statacc[:, 2:4],
                                    gbc_sb[:, 0:2], gbc_sb[:, 2:4], U, O)
            nc.scalar.activation(embed_u[:], y_sb[0][:], AF.Relu,
                                 bias=shc[:, 0:1], scale=scc[:, 0:1])
            ev = embed_v.rearrange("p (c u) -> p c u", c=NC)
            for hf in range(2):
                nc.scalar.activation(ev[:, 4 * hf:4 * hf + 4, :],
                                     yv_all[:, 4 * hf:4 * hf + 4, 0:S],
                                     AF.Relu, bias=shc[:, 1:2],
                                     scale=scc[:, 1:2])

            # ============ score ============
            for r in range(R):
                ps_t1 = psmm.tile([O, S], F32, tag="mm", name="ps_t1")
                nc.tensor.matmul(ps_t1[:], q_sb[:, r, :], embed_u[:],
                                 start=True, stop=True)
                t1 = sc_p.tile([O, S], F16, tag="t1", name="t1")
                nc.vector.tensor_copy(t1[:], ps_t1[:])
                for ch in range(2):
                    out_sb = sc_p.tile([128, V], F16, tag="osb", name="out_sb")
                    for i, (n0, nn) in enumerate(NTILES):
                        ps_sc = pssc.tile([128, 512], F32, tag="sc",
                                          name="ps_sc")
                        nc.tensor.matmul(ps_sc[:, 0:nn],
                                         t1[:, ch * 128:(ch + 1) * 128],
                                         embed_v[:, n0:n0 + nn],
                                         start=True, stop=True)
                        if i % 2 == 0:
                            nc.vector.tensor_copy(out_sb[:, n0:n0 + nn],
                                                  ps_sc[:, 0:nn])
                        else:
                            nc.scalar.copy(out_sb[:, n0:n0 + nn],
                                           ps_sc[:, 0:nn])
                    nc.sync.dma_start(score_d[r, ch * 128:(ch + 1) * 128, :],
                                      out_sb[:])

    nc.compile()
    return nc


def _prep(inputs):
    """Host-side shard/pad/scale/cast/pack. Returns in_maps for 8 cores."""
    def padto(a, n, axis):
        pad = [(0, 0)] * a.ndim
        pad[axis] = (0, n - a.shape[axis])
        return np.pad(a, pad)

    f32 = np.float32
    fu = padto(padto(np.asarray(inputs['feature_u'], f32), UP, 0), UP, 1)
    fv = padto(padto(np.asarray(inputs['feature_v'], f32), UP, 0), UP, 1)
    Mu = padto(padto(np.asarray(inputs['M_u'], f32), UP, 1), UP, 2) * M_SC
    Mv = padto(padto(np.asarray(inputs['M_v'], f32), UP, 1), UP, 2) * M_SC
    W = padto(np.asarray(inputs['W'], f32), UP, 1)
    sfu = padto(np.asarray(inputs['side_feature_u'], f32), UP, 0)
    sfv = padto(np.asarray(inputs['side_feature_v'], f32), UP, 0)

    # catT row order [pre | hidden | side]; reference cat order is
    # [hidden | f@W | side] -> permute w_cat rows to match.
    perm = np.concatenate([np.arange(RH, 2 * RH), np.arange(0, RH),
                           np.arange(2 * RH, 2 * RH + SH)])
    wcat = np.stack(
        [padto(np.asarray(inputs[f'w_cat_{s}'], f32)[perm], 6 * 128, 0)
         for s in ('u', 'v')], 1)                   # [768, 2, 75]
    wcat16 = np.ascontiguousarray(
        wcat.reshape(6, 128, 2, O).transpose(1, 0, 2, 3)).astype(F16N)
    wside = np.stack([np.asarray(inputs['w_side_u'], f32),
                      np.asarray(inputs['w_side_v'], f32)], 1).astype(F16N)
    gbs = np.stack([inputs['g_side_u'], inputs['g_side_v'],
                    inputs['beta_side_u'], inputs['beta_side_v']],
                   1).astype(f32)
    gbc = np.stack([inputs['g_cat_u'], inputs['g_cat_v'],
                    inputs['beta_cat_u'], inputs['beta_cat_v']],
                   1).astype(f32)
    # W repacked to [p, k, r*h] so each k-slice is a contiguous [128, RH] rhs
    w16 = np.ascontiguousarray(
        W.reshape(R, KT, 128, H).transpose(2, 1, 0, 3)).reshape(
        128, KT, RH).astype(F16N)
    q16 = np.ascontiguousarray(
        np.asarray(inputs['Q'], f32).transpose(1, 0, 2)).astype(F16N)

    def pack_f(feat, sl):
        # [2048, 256] rows sl -> [128, 16, 256]: f = k*128 + p
        a = np.ascontiguousarray(feat[sl].T)        # [2048 f, 256]
        return np.ascontiguousarray(
            a.reshape(KT, 128, S).transpose(1, 0, 2)).astype(F16N)

    def pack_m(Msc, r, sl):
        # M[r][:, my rows].T -> [128, 2, 2048]: local row = t*128 + p
        a = np.ascontiguousarray(Msc[r][:, sl].T)   # [256 local, 2048]
        return np.ascontiguousarray(
            a.reshape(2, 128, UP).transpose(1, 0, 2)).astype(F8N)

    in_maps = []
    for c in range(NC):
        sl = slice(c * S, (c + 1) * S)
        in_maps.append({
            "fuT": pack_f(fu, sl),
            "fvT": pack_f(fv, sl),
            "muT": np.stack([pack_m(Mu, r, sl) for r in range(R)]),
            "mvT": np.stack([pack_m(Mv, r, sl) for r in range(R)]),
            "w": w16,
            "q": q16,
            "sfT": np.ascontiguousarray(
                np.stack([sfu[sl].T, sfv[sl].T], 1)).astype(F16N),
            "wside": wside,
            "wcat": wcat16,
            "gb_side": gbs,
            "gb_cat": gbc,
            "ident": np.eye(128, dtype=F16N),
            "mask": np.broadcast_to(
                (np.arange(c * S, (c + 1) * S) < U).astype(F16N),
                (SH, S)).copy(),
        })
    return in_maps


def kernel(**inputs) -> np.ndarray:
    if "nc" not in _CACHE:
        _CACHE["nc"] = _build()
    nc = _CACHE["nc"]
    in_maps = _prep(inputs)
    res = bass_utils.run_bass_kernel_spmd(nc, in_maps, core_ids=list(range(NC)))
    score = np.concatenate(
        [np.asarray(res.results[c]["score"]) for c in range(NC)],
        axis=1).astype(np.float32)
    return score[:, :U, :]


if __name__ == "__main__":
    print("kernel module OK")


# revision 51
# speedup vs baseline: 1.0544x; 1.0005x over previous
"""Trainium2 Bass kernel for nn_GCMC (GNN message passing / GCMC scoring).

v5 strategy: row-shard users AND items across 8 NeuronCores (256 padded rows
each). Message passing is ONE merged ReduceScatter: each core column-shards
M (its 256 v-columns of M_u, u-columns of M_v), computes partial hidden sums
for ALL opposite-side rows from its local projection slice, and a single
ReduceScatter (add) over a [NC, 2, H, R, 258] fp16 payload returns both
sides' per-core hidden rows (side-branch BatchNorm partial sums ride in 2
extra columns). One slim AllGather then shares the pre-BN v-side cat output
y_v plus both sides' cat BatchNorm partial sums, so every core computes
global BatchNorm stats locally and the full embed_v for the final bilinear
score.

Precision: fp16 operands (the PE accumulates in fp32; fp16 measured no worse
than bf16 here), fp32 catT is not needed (fp16 catT/w_cat measured fine), M
and the projection copy it contracts with travel as fp8e4m3 (x64 / x4
scales, undone by the hidden-relu activation scale 2^-8) enabling DoubleRow
matmuls and halving the dominant M DMA traffic. The RS payload is fp16; the
AllGather payload MUST stay fp32 - 16-bit AllGather payloads measurably
degrade (~1.5% extra score error, consistent with the collective
round-tripping 16-bit data through bf16), while the fp16 ReduceScatter only
costs ~0.35%. Measured end-to-end max-rel error: 1.41% vs the 2% gate.

Collectives: 1x fp16 ReduceScatter (23.3us) + 1x fp32 AllGather (30.6us) on
the serial collective device, vs 2 RS + 1 AG = 77us in v2.1. A dummy Sqrt
activation at t=0 preloads the activation-function table so the BatchNorm
Sqrt does not pay a 1.3us table load on the post-RS critical path.
"""
import sys
if '/opt/trn_rl_repo' not in sys.path:
    sys.path.insert(0, '/opt/trn_rl_repo')

import numpy as np
import ml_dtypes

import concourse.bass as bass
import concourse.bacc as bacc
import concourse.mybir as mybir
import concourse.tile as tile
from concourse import bass_utils

F16N = np.float16
F8N = ml_dtypes.float8_e4m3
F32 = mybir.dt.float32
F16 = mybir.dt.float16
FP8 = mybir.dt.float8e4
AF = mybir.ActivationFunctionType
ALU = mybir.AluOpType
AXX = mybir.AxisListType.X
DR = mybir.MatmulPerfMode.DoubleRow

U = V = F = 2000
R, H, O, SH, SF = 5, 64, 75, 64, 128
RH = R * H           # 320
UP = 2048            # padded U/V/F
S = 256              # rows per core
SP2 = S + 2          # RS payload row width (256 data + 2 BN-sum cols)
NC = 8
KT = 16              # 128-row k-tiles over the padded 2048 contraction dims
EPS = 1e-5
M_SC, P_SC = 64.0, 4.0          # fp8 scales for M and prevh
HID_SC = 1.0 / (M_SC * P_SC)    # 2^-8, folded into hidden relu
NTILES = [(0, 512), (512, 512), (1024, 512), (1536, 464)]  # score v-tiles

_CACHE = {}


def _build():
    nc = bacc.Bacc("TRN2", target_bir_lowering=False, debug=False,
                   num_devices=NC)

    def din(name, shape, dt):
        return nc.dram_tensor(name, list(shape), dt, kind="ExternalInput").ap()

    fuT_d = din("fuT", (128, KT, S), F16)     # my u rows, [f, kt, u]
    fvT_d = din("fvT", (128, KT, S), F16)
    muT_d = din("muT", (R, 128, 2, UP), FP8)  # x64 M_u[r][:, my_v].T packed
    mvT_d = din("mvT", (R, 128, 2, UP), FP8)
    w_d = din("w", (128, KT, RH), F16)
    q_d = din("q", (O, R, O), F16)
    sfT_d = din("sfT", (SF, 2, S), F16)
    wside_d = din("wside", (SF, 2, SH), F16)
    wcat_d = din("wcat", (128, 6, 2, O), F16)  # rows: pre|hidden|side
    gbs_d = din("gb_side", (SH, 4), F32)
    gbc_d = din("gb_cat", (O, 4), F32)
    ident_d = din("ident", (128, 128), F16)
    mask_d = din("mask", (SH, S), F16)

    score_d = nc.dram_tensor("score", [R, S, V], F16,
                             kind="ExternalOutput").ap()

    with tile.TileContext(nc) as tc:
        with tc.tile_pool(name="const", bufs=1) as const_p, \
             tc.tile_pool(name="big", bufs=1) as big_p, \
             tc.tile_pool(name="mring", bufs=10) as m_p, \
             tc.tile_pool(name="small", bufs=1) as sm_p, \
             tc.tile_pool(name="scoresb", bufs=3) as sc_p, \
             tc.tile_pool(name="psmm", bufs=4, space="PSUM") as psmm, \
             tc.tile_pool(name="pssc", bufs=3, space="PSUM") as pssc, \
             tc.tile_pool(name="dram", bufs=1, space="DRAM") as dram_p:

            # ============ constant/small loads (SP queue) ============
            ident = const_p.tile([128, 128], F16)
            nc.sync.dma_start(ident[:], ident_d)
            eps_t = const_p.tile([128, 1], F32)
            nc.vector.memset(eps_t[:], EPS)
            sqrt_warm = const_p.tile([128, 1], F32, name="sqrt_warm")
            nc.scalar.activation(sqrt_warm[:], eps_t[:], AF.Sqrt,
                                 bias=eps_t[:])
            sfT_sb = const_p.tile([SF, 2, S], F16)
            nc.sync.dma_start(sfT_sb[:], sfT_d)
            wside_sb = const_p.tile([SF, 2, SH], F16)
            nc.sync.dma_start(wside_sb[:], wside_d)
            q_sb = const_p.tile([O, R, O], F16)
            nc.sync.dma_start(q_sb[:], q_d)
            wcat_sb = const_p.tile([128, 6, 2, O], F16)
            nc.sync.dma_start(wcat_sb[:], wcat_d)
            gbs_sb = const_p.tile([SH, 4], F32)
            nc.sync.dma_start(gbs_sb[:], gbs_d)
            gbc_sb = const_p.tile([O, 4], F32)
            nc.sync.dma_start(gbc_sb[:], gbc_d)
            mask_sb = const_p.tile([SH, S], F16)
            nc.sync.dma_start(mask_sb[:], mask_d)

            # ============ big stream (ACT queue, exact order) ============
            w_sb = big_p.tile([128, KT, RH], F16)
            nc.scalar.dma_start(w_sb[:], w_d)
            fvT_sb = big_p.tile([128, KT, S], F16)
            nc.scalar.dma_start(fvT_sb[:, :, 0:128], fvT_d[:, :, 0:128])
            nc.scalar.dma_start(fvT_sb[:, :, 128:S], fvT_d[:, :, 128:S])
            fuT_sb = big_p.tile([128, KT, S], F16)
            nc.scalar.dma_start(fuT_sb[:], fuT_d)
            muT_sb = [m_p.tile([128, 2, UP], FP8, tag="mT", name=f"muT_{r}")
                      for r in range(R)]
            mvT_sb = [m_p.tile([128, 2, UP], FP8, tag="mT", name=f"mvT_{r}")
                      for r in range(R)]
            for r in range(R):
                nc.scalar.dma_start(muT_sb[r][:], muT_d[r])
            for r in range(R):
                nc.scalar.dma_start(mvT_sb[r][:], mvT_d[r])

            # ============ collective buffers ============
            replica = [list(range(NC))]
            rs_in = dram_p.tile([NC, 2, H, R, SP2], F16)
            rs_out = dram_p.tile([2, H, R, SP2], F16)
            ag_in = dram_p.tile([O, S + 4], F32)
            ag_out = dram_p.tile([NC, O, S + 4], F32, addr_space="Shared")

            # catT: [128, 6, S] f16 per side; rows pre(0:320)|hidden|side
            catT = [big_p.tile([128, 6, S], F16, name=f"catT{sd}")
                    for sd in range(2)]
            for sd in range(2):
                nc.vector.memset(catT[sd][SH:128, 5, :], 0.0)

            # partial-hidden staging, one tile per side (avoids WAR
            # serialization of v-side copies behind u-side stage DMAs)
            stages = [big_p.tile([H, NC, R, SP2], F16, name=f"stage{sd}")
                      for sd in range(2)]
            for sd in range(2):
                nc.vector.memset(stages[sd][:, :, :, S:SP2], 0.0)

            # ============ side matmuls + BN partial sums ============
            s_loc = sm_p.tile([SH, 2, S], F32)
            junk = sm_p.tile([128, S], F32, name="junk")

            s_sums = [sm_p.tile([SH, 2], F32, name=f"s_sums{sd}")
                      for sd in range(2)]

            def side_compute(sd):
                ps_s = psmm.tile([SH, S], F32, tag="mm", name="ps_side")
                nc.tensor.matmul(ps_s[:], wside_sb[:, sd, :], sfT_sb[:, sd, :],
                                 start=True, stop=True)
                nc.vector.tensor_copy(s_loc[:, sd, :], ps_s[:])
                nc.vector.reduce_sum(s_sums[sd][:, 0:1], s_loc[:, sd, :],
                                     axis=AXX)
                nc.vector.tensor_mul(junk[0:SH, :], s_loc[:, sd, :],
                                     s_loc[:, sd, :])
                nc.vector.reduce_sum(s_sums[sd][:, 1:2], junk[0:SH, :],
                                     axis=AXX)

            def side_stage(sd):
                # replicate into every dest shard of the RS payload (row r=0)
                for c in range(NC):
                    nc.vector.tensor_copy(stages[sd][:, c, 0, S:S + 2],
                                          s_sums[sd][:])

            # ============ projection: pre[row, rh] = f^T W ============
            prevh = [big_p.tile([128, 2, RH], F16, name=f"prevh_{sd}")
                     for sd in range(2)]
            prevh8 = [big_p.tile([128, 2, RH], FP8, name=f"prevh8_{sd}")
                      for sd in range(2)]

            def proj_side(sd, fT):
                pre_f16 = prevh[sd]
                for ch in range(2):
                    ps_pre = psmm.tile([128, RH], F32, tag="mm", name="ps_pre")
                    for k in range(KT):
                        nc.tensor.matmul(ps_pre[:],
                                         fT[:, k, ch * 128:(ch + 1) * 128],
                                         w_sb[:, k, :],
                                         start=(k == 0), stop=(k == KT - 1))
                    nc.vector.tensor_copy(pre_f16[:, ch, :], ps_pre[:])
                    nc.scalar.activation(prevh8[sd][:, ch, :], ps_pre[:],
                                         AF.Copy, scale=P_SC)

            # deferred: transpose prevh into catT pre rows during the RS
            # window (keeps these copies off the pre-RS DVE critical path)
            def catT_pre_fill(sd):
                for ch in range(2):
                    for c in range(3):
                        cw = min(128, RH - c * 128)
                        ps_t = psmm.tile([128, 128], F16, tag="mm",
                                         name="ps_t")
                        nc.tensor.transpose(
                            ps_t[0:cw, :],
                            prevh[sd][:, ch, c * 128:c * 128 + cw],
                            ident[:])
                        row = c * 128
                        blk, off = divmod(row, 128)
                        eng = nc.vector if (ch + c) % 2 == 0 else None
                        if eng is None:
                            nc.scalar.copy(
                                catT[sd][off:off + cw, blk,
                                         ch * 128:(ch + 1) * 128],
                                ps_t[0:cw, :])
                        else:
                            nc.vector.tensor_copy(
                                catT[sd][off:off + cw, blk,
                                         ch * 128:(ch + 1) * 128],
                                ps_t[0:cw, :])

            # ==== partial hidden (fp8 DoubleRow) -> f16 stage -> DMA ====
            def partial_side(sd, mT, osd):
                stage = stages[sd]
                for r in range(R):
                    pss = [psmm.tile([H, 2, S], F32, tag="mm", name=f"ps_p{g}")
                           for g in range(4)]
                    for g in range(4):
                        for c in range(2):
                            nc.tensor.matmul(
                                pss[g][:, c, :],
                                prevh8[osd][:, :, r * H:(r + 1) * H],
                                mT[r][:, :, (2 * g + c) * S:(2 * g + c + 1) * S],
                                start=True, stop=True, perf_mode=DR)
                    for g in range(4):
                        dst = stage[:, 2 * g:2 * g + 2, r, 0:S]
                        if g % 2 == 0:
                            nc.vector.tensor_copy(dst, pss[g][:])
                        else:
                            nc.scalar.copy(dst, pss[g][:])
                    nc.sync.dma_start(
                        rs_in[:, sd, :, r, :].rearrange("c h x -> h c x"),
                        stage[:, :, r, :])

            side_compute(0)
            side_compute(1)
            proj_side(1, fvT_sb)
            proj_side(0, fuT_sb)
            side_stage(0)
            partial_side(0, muT_sb, 1)
            side_stage(1)
            partial_side(1, mvT_sb, 0)
            catT_pre_fill(1)
            catT_pre_fill(0)
            nc.gpsimd.collective_compute("ReduceScatter", ALU.add,
                                         replica_groups=replica,
                                         ins=[rs_in.opt()],
                                         outs=[rs_out.opt()])



            # ============ BN helpers (both sides batched: [P, 2]) ======
            def bn_from_sums(tg, sums, sumsq, g_col, b_col, n, P):
                def t(nm):
                    return sm_p.tile([P, 2], F32, tag=f"{nm}_{tg}",
                                     name=f"{nm}_{tg}")
                mu = t("bn_mu")
                nc.vector.tensor_scalar_mul(mu[:], sums[:], 1.0 / n)
                e2 = t("bn_e2")
                nc.vector.tensor_scalar_mul(e2[:], sumsq[:], 1.0 / n)
                var = t("bn_var")
                nc.vector.tensor_mul(var[:], mu[:], mu[:])
                nc.vector.tensor_sub(var[:], e2[:], var[:])
                std = t("bn_std")
                nc.scalar.activation(std[:], var[:], AF.Sqrt, bias=eps_t[0:P, :])
                rstd = t("bn_rstd")
                nc.vector.reciprocal(rstd[:], std[:])
                scale = t("bn_scale")
                nc.vector.tensor_mul(scale[:], g_col, rstd[:])
                shift = t("bn_shift")
                nc.vector.tensor_mul(shift[:], mu[:], scale[:])
                nc.vector.tensor_sub(shift[:], b_col, shift[:])
                return scale, shift

            # ====== hidden relu into catT rows 320:640; side BN ======
            side_tmp = sm_p.tile([SH, 2, S], F16, name="side_tmp")
            hsum = sm_p.tile([H, 2, R, SP2], F16, name="hsum")
            nc.sync.dma_start(hsum[:], rs_out.rearrange("s h r x -> h s r x"))
            t_sums = sm_p.tile([SH, 2, 2], F32, name="t_sums")
            nc.vector.tensor_copy(t_sums[:], hsum[:, :, 0, S:S + 2])
            for sd in range(2):
                for r in range(R):
                    row = RH + r * H
                    blk, off = divmod(row, 128)
                    if r % 2 == 0:
                        nc.scalar.activation(catT[sd][off:off + H, blk, :],
                                             hsum[:, sd, r, 0:S],
                                             AF.Relu, scale=HID_SC)
                    else:
                        nc.vector.tensor_scalar(
                            catT[sd][off:off + H, blk, :], hsum[:, sd, r, 0:S],
                            HID_SC, 0.0, op0=ALU.mult, op1=ALU.max)
            sc2, sh2 = bn_from_sums("sB", t_sums[:, :, 0], t_sums[:, :, 1],
                                    gbs_sb[:, 0:2], gbs_sb[:, 2:4], U, SH)
            for sd in range(2):
                nc.scalar.activation(side_tmp[:, sd, :], s_loc[:, sd, :],
                                     AF.Relu, bias=sh2[:, sd:sd + 1],
                                     scale=sc2[:, sd:sd + 1])
                nc.vector.tensor_mul(catT[sd][0:SH, 5, :], side_tmp[:, sd, :],
                                     mask_sb[:])

            # ============ cat matmul (f16) + slim AG ============
            y_sb = [sm_p.tile([O, S], F32, name=f"y_sb{sd}") for sd in range(2)]
            stats = sm_p.tile([O, 4], F32, name="stats")

            for sd in range(2):
                ps_y = psmm.tile([O, S], F32, tag="mm", name="ps_y")
                for b in range(6):
                    nc.tensor.matmul(ps_y[:], wcat_sb[:, b, sd, :],
                                     catT[sd][:, b, :],
                                     start=(b == 0), stop=(b == 5))
                nc.vector.tensor_copy(y_sb[sd][:], ps_y[:])
                nc.vector.reduce_sum(stats[:, sd:sd + 1], y_sb[sd][:],
                                     axis=AXX)
                nc.vector.tensor_mul(junk[0:O, :], y_sb[sd][:],
                                     y_sb[sd][:])
                nc.vector.reduce_sum(stats[:, 2 + sd:3 + sd],
                                     junk[0:O, :], axis=AXX)
            ag_st = sm_p.tile([O, S + 4], F32, name="ag_st")
            nc.vector.tensor_copy(ag_st[:, 0:S], y_sb[1][:])
            nc.vector.tensor_copy(ag_st[:, S:S + 4], stats[:])
            nc.sync.dma_start(ag_in[:], ag_st[:])
            nc.gpsimd.collective_compute("AllGather", ALU.bypass,
                                         replica_groups=replica,
                                         ins=[ag_in.opt()],
                                         outs=[ag_out.opt()])

            yv_all = sm_p.tile([O, NC, S + 4], F32, name="yv_all")
            nc.sync.dma_start(yv_all[:, :, S:S + 4],
                              ag_out[:, :, S:S + 4].rearrange("c p x -> p c x"))
            nc.sync.dma_start(yv_all[:, 0:4, 0:S],
                              ag_out[0:4, :, 0:S].rearrange("c p x -> p c x"))
            nc.sync.dma_start(yv_all[:, 4:8, 0:S],
                              ag_out[4:8, :, 0:S].rearrange("c p x -> p c x"))

            # ============ cat BN (global stats) + embeds ============
            statacc = sm_p.tile([O, 4], F32, name="statacc")
            stat_b = sm_p.tile([O, 4], F32, name="stat_b")
            nc.vector.tensor_add(statacc[:], yv_all[:, 0, S:S + 4],
                                 yv_all[:, 1, S:S + 4])
            nc.gpsimd.tensor_add(stat_b[:], yv_all[:, 4, S:S + 4],
                                 yv_all[:, 5, S:S + 4])
            for c in (2, 3):
                nc.vector.tensor_add(statacc[:], statacc[:],
                                     yv_all[:, c, S:S + 4])
            for c in (6, 7):
                nc.gpsimd.tensor_add(stat_b[:], stat_b[:],
                                     yv_all[:, c, S:S + 4])
            nc.vector.tensor_add(statacc[:], statacc[:], stat_b[:])
            embed_u = sm_p.tile([O, S], F16)
            embed_v = sm_p.tile([O, UP], F16)
            scc, shc = bn_from_sums("cB", statacc[:, 0:2], statacc[:, 2:4],
                                    gbc_sb[:, 0:2], gbc_sb[:, 2:4], U, O)
            nc.scalar.activation(embed_u[:], y_sb[0][:], AF.Relu,
                                 bias=shc[:, 0:1], scale=scc[:, 0:1])
            ev = embed_v.rearrange("p (c u) -> p c u", c=NC)
            nc.scalar.activation(ev[:, 0:4, :], yv_all[:, 0:4, 0:S],
                                 AF.Relu, bias=shc[:, 1:2],
                                 scale=scc[:, 1:2])
            ev_t = sm_p.tile([O, 4, S], F32, name="ev_t")
            nc.vector.tensor_scalar(ev_t[:], yv_all[:, 4:8, 0:S],
                                    scc[:, 1:2], shc[:, 1:2],
                                    op0=ALU.mult, op1=ALU.add)
            nc.vector.tensor_scalar(ev[:, 4:8, :], ev_t[:], 0.0, None,
                                    op0=ALU.max)

            # ============ score ============
            for r in range(R):
                ps_t1 = psmm.tile([O, S], F32, tag="mm", name="ps_t1")
                nc.tensor.matmul(ps_t1[:], q_sb[:, r, :], embed_u[:],
                                 start=True, stop=True)
                t1 = sc_p.tile([O, S], F16, tag="t1", name="t1")
                nc.vector.tensor_copy(t1[:], ps_t1[:])
                for ch in range(2):
                    out_sb = sc_p.tile([128, V], F16, tag="osb", name="out_sb")
                    for i, (n0, nn) in enumerate(NTILES):
                        ps_sc = pssc.tile([128, 512], F32, tag="sc",
                                          name="ps_sc")
                        nc.tensor.matmul(ps_sc[:, 0:nn],
                                         t1[:, ch * 128:(ch + 1) * 128],
                                         embed_v[:, n0:n0 + nn],
                                         start=True, stop=True)
                        if i % 2 == 0:
                            nc.vector.tensor_copy(out_sb[:, n0:n0 + nn],
                                                  ps_sc[:, 0:nn])
                        else:
                            nc.scalar.copy(out_sb[:, n0:n0 + nn],
                                           ps_sc[:, 0:nn])
                    nc.sync.dma_start(score_d[r, ch * 128:(ch + 1) * 128, :],
                                      out_sb[:])

    nc.compile()
    return nc


def _prep(inputs):
    """Host-side shard/pad/scale/cast/pack. Returns in_maps for 8 cores."""
    def padto(a, n, axis):
        pad = [(0, 0)] * a.ndim
        pad[axis] = (0, n - a.shape[axis])
        return np.pad(a, pad)

    f32 = np.float32
    fu = padto(padto(np.asarray(inputs['feature_u'], f32), UP, 0), UP, 1)
    fv = padto(padto(np.asarray(inputs['feature_v'], f32), UP, 0), UP, 1)
    Mu = padto(padto(np.asarray(inputs['M_u'], f32), UP, 1), UP, 2) * M_SC
    Mv = padto(padto(np.asarray(inputs['M_v'], f32), UP, 1), UP, 2) * M_SC
    W = padto(np.asarray(inputs['W'], f32), UP, 1)
    sfu = padto(np.asarray(inputs['side_feature_u'], f32), UP, 0)
    sfv = padto(np.asarray(inputs['side_feature_v'], f32), UP, 0)

    # catT row order [pre | hidden | side]; reference cat order is
    # [hidden | f@W | side] -> permute w_cat rows to match.
    perm = np.concatenate([np.arange(RH, 2 * RH), np.arange(0, RH),
                           np.arange(2 * RH, 2 * RH + SH)])
    wcat = np.stack(
        [padto(np.asarray(inputs[f'w_cat_{s}'], f32)[perm], 6 * 128, 0)
         for s in ('u', 'v')], 1)                   # [768, 2, 75]
    wcat16 = np.ascontiguousarray(
        wcat.reshape(6, 128, 2, O).transpose(1, 0, 2, 3)).astype(F16N)
    wside = np.stack([np.asarray(inputs['w_side_u'], f32),
                      np.asarray(inputs['w_side_v'], f32)], 1).astype(F16N)
    gbs = np.stack([inputs['g_side_u'], inputs['g_side_v'],
                    inputs['beta_side_u'], inputs['beta_side_v']],
                   1).astype(f32)
    gbc = np.stack([inputs['g_cat_u'], inputs['g_cat_v'],
                    inputs['beta_cat_u'], inputs['beta_cat_v']],
                   1).astype(f32)
    # W repacked to [p, k, r*h] so each k-slice is a contiguous [128, RH] rhs
    w16 = np.ascontiguousarray(
        W.reshape(R, KT, 128, H).transpose(2, 1, 0, 3)).reshape(
        128, KT, RH).astype(F16N)
    q16 = np.ascontiguousarray(
        np.asarray(inputs['Q'], f32).transpose(1, 0, 2)).astype(F16N)

    def pack_f(feat, sl):
        # [2048, 256] rows sl -> [128, 16, 256]: f = k*128 + p
        a = np.ascontiguousarray(feat[sl].T)        # [2048 f, 256]
        return np.ascontiguousarray(
            a.reshape(KT, 128, S).transpose(1, 0, 2)).astype(F16N)

    def pack_m(Msc, r, sl):
        # M[r][:, my rows].T -> [128, 2, 2048]: local row = t*128 + p
        a = np.ascontiguousarray(Msc[r][:, sl].T)   # [256 local, 2048]
        return np.ascontiguousarray(
            a.reshape(2, 128, UP).transpose(1, 0, 2)).astype(F8N)

    in_maps = []
    for c in range(NC):
        sl = slice(c * S, (c + 1) * S)
        in_maps.append({
            "fuT": pack_f(fu, sl),
            "fvT": pack_f(fv, sl),
            "muT": np.stack([pack_m(Mu, r, sl) for r in range(R)]),
            "mvT": np.stack([pack_m(Mv, r, sl) for r in range(R)]),
            "w": w16,
            "q": q16,
            "sfT": np.ascontiguousarray(
                np.stack([sfu[sl].T, sfv[sl].T], 1)).astype(F16N),
            "wside": wside,
            "wcat": wcat16,
            "gb_side": gbs,
            "gb_cat": gbc,
            "ident": np.eye(128, dtype=F16N),
            "mask": np.broadcast_to(
                (np.arange(c * S, (c + 1) * S) < U).astype(F16N),
                (SH, S)).copy(),
        })
    return in_maps


def kernel(**inputs) -> np.ndarray:
    if "nc" not in _CACHE:
        _CACHE["nc"] = _build()
    nc = _CACHE["nc"]
    in_maps = _prep(inputs)
    res = bass_utils.run_bass_kernel_spmd(nc, in_maps, core_ids=list(range(NC)))
    score = np.concatenate(
        [np.asarray(res.results[c]["score"]) for c in range(NC)],
        axis=1).astype(np.float32)
    return score[:, :U, :]


if __name__ == "__main__":
    print("kernel module OK")


# revision 53
# speedup vs baseline: 1.0601x; 1.0054x over previous
"""Trainium2 Bass kernel for nn_GCMC (GNN message passing / GCMC scoring).

v5 strategy: row-shard users AND items across 8 NeuronCores (256 padded rows
each). Message passing is ONE merged ReduceScatter: each core column-shards
M (its 256 v-columns of M_u, u-columns of M_v), computes partial hidden sums
for ALL opposite-side rows from its local projection slice, and a single
ReduceScatter (add) over a [NC, 2, H, R, 258] fp16 payload returns both
sides' per-core hidden rows (side-branch BatchNorm partial sums ride in 2
extra columns). One slim AllGather then shares the pre-BN v-side cat output
y_v plus both sides' cat BatchNorm partial sums, so every core computes
global BatchNorm stats locally and the full embed_v for the final bilinear
score.

Precision: fp16 operands (the PE accumulates in fp32; fp16 measured no worse
than bf16 here), fp32 catT is not needed (fp16 catT/w_cat measured fine), M
and the projection copy it contracts with travel as fp8e4m3 (x64 / x4
scales, undone by the hidden-relu activation scale 2^-8) enabling DoubleRow
matmuls and halving the dominant M DMA traffic. The RS payload is fp16; the
AllGather payload MUST stay fp32 - 16-bit AllGather payloads measurably
degrade (~1.5% extra score error, consistent with the collective
round-tripping 16-bit data through bf16), while the fp16 ReduceScatter only
costs ~0.35%. Measured end-to-end max-rel error: 1.41% vs the 2% gate.

Collectives: 1x fp16 ReduceScatter (23.3us) + 1x fp32 AllGather (30.6us) on
the serial collective device, vs 2 RS + 1 AG = 77us in v2.1. A dummy Sqrt
activation at t=0 preloads the activation-function table so the BatchNorm
Sqrt does not pay a 1.3us table load on the post-RS critical path.
"""
import sys
if '/opt/trn_rl_repo' not in sys.path:
    sys.path.insert(0, '/opt/trn_rl_repo')

import numpy as np
import ml_dtypes

import concourse.bass as bass
import concourse.bacc as bacc
import concourse.mybir as mybir
import concourse.tile as tile
from concourse import bass_utils

F16N = np.float16
F8N = ml_dtypes.float8_e4m3
F32 = mybir.dt.float32
F16 = mybir.dt.float16
FP8 = mybir.dt.float8e4
AF = mybir.ActivationFunctionType
ALU = mybir.AluOpType
AXX = mybir.AxisListType.X
DR = mybir.MatmulPerfMode.DoubleRow

U = V = F = 2000
R, H, O, SH, SF = 5, 64, 75, 64, 128
RH = R * H           # 320
UP = 2048            # padded U/V/F
S = 256              # rows per core
SP2 = S + 2          # RS payload row width (256 data + 2 BN-sum cols)
NC = 8
KT = 16              # 128-row k-tiles over the padded 2048 contraction dims
EPS = 1e-5
M_SC, P_SC = 64.0, 4.0          # fp8 scales for M and prevh
HID_SC = 1.0 / (M_SC * P_SC)    # 2^-8, folded into hidden relu
NTILES = [(0, 512), (512, 512), (1024, 512), (1536, 464)]  # score v-tiles

_CACHE = {}


def _build():
    nc = bacc.Bacc("TRN2", target_bir_lowering=False, debug=False,
                   num_devices=NC)

    def din(name, shape, dt):
        return nc.dram_tensor(name, list(shape), dt, kind="ExternalInput").ap()

    fuT_d = din("fuT", (128, KT, S), F16)     # my u rows, [f, kt, u]
    fvT_d = din("fvT", (128, KT, S), F16)
    muT_d = din("muT", (R, 128, 2, UP), FP8)  # x64 M_u[r][:, my_v].T packed
    mvT_d = din("mvT", (R, 128, 2, UP), FP8)
    w_d = din("w", (128, KT, RH), F16)
    q_d = din("q", (O, R, O), F16)
    sfT_d = din("sfT", (SF, 2, S), F16)
    wside_d = din("wside", (SF, 2, SH), F16)
    wcat_d = din("wcat", (128, 6, 2, O), F16)  # rows: pre|hidden|side
    gbs_d = din("gb_side", (SH, 4), F32)
    gbc_d = din("gb_cat", (O, 4), F32)
    ident_d = din("ident", (128, 128), F16)
    mask_d = din("mask", (SH, S), F16)

    score_d = nc.dram_tensor("score", [R, S, V], F16,
                             kind="ExternalOutput").ap()

    with tile.TileContext(nc) as tc:
        with tc.tile_pool(name="const", bufs=1) as const_p, \
             tc.tile_pool(name="big", bufs=1) as big_p, \
             tc.tile_pool(name="mring", bufs=10) as m_p, \
             tc.tile_pool(name="small", bufs=1) as sm_p, \
             tc.tile_pool(name="scoresb", bufs=3) as sc_p, \
             tc.tile_pool(name="psmm", bufs=4, space="PSUM") as psmm, \
             tc.tile_pool(name="pssc", bufs=3, space="PSUM") as pssc, \
             tc.tile_pool(name="dram", bufs=1, space="DRAM") as dram_p:

            # ============ constant/small loads (SP queue) ============
            ident = const_p.tile([128, 128], F16)
            nc.sync.dma_start(ident[:], ident_d)
            eps_t = const_p.tile([128, 1], F32)
            nc.vector.memset(eps_t[:], EPS)
            sqrt_warm = const_p.tile([128, 1], F32, name="sqrt_warm")
            nc.scalar.activation(sqrt_warm[:], eps_t[:], AF.Sqrt,
                                 bias=eps_t[:])
            sfT_sb = const_p.tile([SF, 2, S], F16)
            nc.sync.dma_start(sfT_sb[:], sfT_d)
            wside_sb = const_p.tile([SF, 2, SH], F16)
            nc.sync.dma_start(wside_sb[:], wside_d)
            q_sb = const_p.tile([O, R, O], F16)
            nc.sync.dma_start(q_sb[:], q_d)
            wcat_sb = const_p.tile([128, 6, 2, O], F16)
            nc.sync.dma_start(wcat_sb[:], wcat_d)
            gbs_sb = const_p.tile([SH, 4], F32)
            nc.sync.dma_start(gbs_sb[:], gbs_d)
            gbc_sb = const_p.tile([O, 4], F32)
            nc.sync.dma_start(gbc_sb[:], gbc_d)
            mask_sb = const_p.tile([SH, S], F16)
            nc.sync.dma_start(mask_sb[:], mask_d)

            # ============ big stream (ACT queue, exact order) ============
            w_sb = big_p.tile([128, KT, RH], F16)
            nc.scalar.dma_start(w_sb[:], w_d)
            fvT_sb = big_p.tile([128, KT, S], F16)
            nc.scalar.dma_start(fvT_sb[:, :, 0:128], fvT_d[:, :, 0:128])
            nc.scalar.dma_start(fvT_sb[:, :, 128:S], fvT_d[:, :, 128:S])
            fuT_sb = big_p.tile([128, KT, S], F16)
            nc.scalar.dma_start(fuT_sb[:], fuT_d)
            muT_sb = [m_p.tile([128, 2, UP], FP8, tag="mT", name=f"muT_{r}")
                      for r in range(R)]
            mvT_sb = [m_p.tile([128, 2, UP], FP8, tag="mT", name=f"mvT_{r}")
                      for r in range(R)]
            for r in range(R):
                nc.scalar.dma_start(muT_sb[r][:], muT_d[r])
            for r in range(R):
                nc.scalar.dma_start(mvT_sb[r][:], mvT_d[r])

            # ============ collective buffers ============
            replica = [list(range(NC))]
            rs_in = dram_p.tile([NC, 2, H, R, SP2], F16)
            rs_out = dram_p.tile([2, H, R, SP2], F16)
            ag_in = dram_p.tile([O, S + 4], F32)
            ag_out = dram_p.tile([NC, O, S + 4], F32, addr_space="Shared")

            # catT: [128, 6, S] f16 per side; rows pre(0:320)|hidden|side
            catT = [big_p.tile([128, 6, S], F16, name=f"catT{sd}")
                    for sd in range(2)]
            for sd in range(2):
                nc.vector.memset(catT[sd][SH:128, 5, :], 0.0)

            # partial-hidden staging, one tile per side (avoids WAR
            # serialization of v-side copies behind u-side stage DMAs)
            stages = [big_p.tile([H, NC, R, SP2], F16, name=f"stage{sd}")
                      for sd in range(2)]
            for sd in range(2):
                nc.vector.memset(stages[sd][:, :, :, S:SP2], 0.0)

            # ============ side matmuls + BN partial sums ============
            s_loc = sm_p.tile([SH, 2, S], F32)
            junk = sm_p.tile([128, S], F32, name="junk")

            s_sums = [sm_p.tile([SH, 2], F32, name=f"s_sums{sd}")
                      for sd in range(2)]

            def side_compute(sd):
                ps_s = psmm.tile([SH, S], F32, tag="mm", name="ps_side")
                nc.tensor.matmul(ps_s[:], wside_sb[:, sd, :], sfT_sb[:, sd, :],
                                 start=True, stop=True)
                nc.vector.tensor_copy(s_loc[:, sd, :], ps_s[:])
                nc.vector.reduce_sum(s_sums[sd][:, 0:1], s_loc[:, sd, :],
                                     axis=AXX)
                nc.vector.tensor_mul(junk[0:SH, :], s_loc[:, sd, :],
                                     s_loc[:, sd, :])
                nc.vector.reduce_sum(s_sums[sd][:, 1:2], junk[0:SH, :],
                                     axis=AXX)

            def side_stage(sd):
                # replicate into every dest shard of the RS payload (row r=0)
                for c in range(NC):
                    nc.vector.tensor_copy(stages[sd][:, c, 0, S:S + 2],
                                          s_sums[sd][:])

            # ============ projection: pre[row, rh] = f^T W ============
            prevh = [big_p.tile([128, 2, RH], F16, name=f"prevh_{sd}")
                     for sd in range(2)]
            prevh8 = [big_p.tile([128, 2, RH], FP8, name=f"prevh8_{sd}")
                      for sd in range(2)]

            def proj_side(sd, fT):
                pre_f16 = prevh[sd]
                for ch in range(2):
                    ps_pre = psmm.tile([128, RH], F32, tag="mm", name="ps_pre")
                    for k in range(KT):
                        nc.tensor.matmul(ps_pre[:],
                                         fT[:, k, ch * 128:(ch + 1) * 128],
                                         w_sb[:, k, :],
                                         start=(k == 0), stop=(k == KT - 1))
                    nc.vector.tensor_copy(pre_f16[:, ch, :], ps_pre[:])
                    nc.scalar.activation(prevh8[sd][:, ch, :], ps_pre[:],
                                         AF.Copy, scale=P_SC)

            # deferred: transpose prevh into catT pre rows during the RS
            # window (keeps these copies off the pre-RS DVE critical path)
            def catT_pre_fill(sd):
                for ch in range(2):
                    for c in range(3):
                        cw = min(128, RH - c * 128)
                        ps_t = psmm.tile([128, 128], F16, tag="mm",
                                         name="ps_t")
                        nc.tensor.transpose(
                            ps_t[0:cw, :],
                            prevh[sd][:, ch, c * 128:c * 128 + cw],
                            ident[:])
                        row = c * 128
                        blk, off = divmod(row, 128)
                        eng = nc.vector if (ch + c) % 2 == 0 else None
                        if eng is None:
                            nc.scalar.copy(
                                catT[sd][off:off + cw, blk,
                                         ch * 128:(ch + 1) * 128],
                                ps_t[0:cw, :])
                        else:
                            nc.vector.tensor_copy(
                                catT[sd][off:off + cw, blk,
                                         ch * 128:(ch + 1) * 128],
                                ps_t[0:cw, :])

            # ==== partial hidden (fp8 DoubleRow) -> f16 stage -> DMA ====
            def partial_side(sd, mT, osd):
                stage = stages[sd]
                for r in range(R):
                    pss = [psmm.tile([H, 2, S], F32, tag="mm", name=f"ps_p{g}")
                           for g in range(4)]
                    for g in range(4):
                        for c in range(2):
                            nc.tensor.matmul(
                                pss[g][:, c, :],
                                prevh8[osd][:, :, r * H:(r + 1) * H],
                                mT[r][:, :, (2 * g + c) * S:(2 * g + c + 1) * S],
                                start=True, stop=True, perf_mode=DR)
                    for g in range(4):
                        dst = stage[:, 2 * g:2 * g + 2, r, 0:S]
                        if g % 2 == 0:
                            nc.vector.tensor_copy(dst, pss[g][:])
                        else:
                            nc.scalar.copy(dst, pss[g][:])
                    nc.sync.dma_start(
                        rs_in[:, sd, :, r, :].rearrange("c h x -> h c x"),
                        stage[:, :, r, :])

            side_compute(0)
            side_compute(1)
            proj_side(1, fvT_sb)
            proj_side(0, fuT_sb)
            side_stage(0)
            partial_side(0, muT_sb, 1)
            side_stage(1)
            partial_side(1, mvT_sb, 0)
            catT_pre_fill(1)
            catT_pre_fill(0)
            nc.gpsimd.collective_compute("ReduceScatter", ALU.add,
                                         replica_groups=replica,
                                         ins=[rs_in.opt()],
                                         outs=[rs_out.opt()])



            # ============ BN helpers (both sides batched: [P, 2]) ======
            def bn_from_sums(tg, sums, sumsq, g_col, b_col, n, P):
                def t(nm):
                    return sm_p.tile([P, 2], F32, tag=f"{nm}_{tg}",
                                     name=f"{nm}_{tg}")
                mu = t("bn_mu")
                nc.vector.tensor_scalar_mul(mu[:], sums[:], 1.0 / n)
                e2 = t("bn_e2")
                nc.vector.tensor_scalar_mul(e2[:], sumsq[:], 1.0 / n)
                var = t("bn_var")
                nc.vector.tensor_mul(var[:], mu[:], mu[:])
                nc.vector.tensor_sub(var[:], e2[:], var[:])
                std = t("bn_std")
                nc.scalar.activation(std[:], var[:], AF.Sqrt, bias=eps_t[0:P, :])
                rstd = t("bn_rstd")
                nc.vector.reciprocal(rstd[:], std[:])
                scale = t("bn_scale")
                nc.vector.tensor_mul(scale[:], g_col, rstd[:])
                shift = t("bn_shift")
                nc.vector.tensor_mul(shift[:], mu[:], scale[:])
                nc.vector.tensor_sub(shift[:], b_col, shift[:])
                return scale, shift

            # ====== hidden relu into catT rows 320:640; side BN ======
            side_tmp = sm_p.tile([SH, 2, S], F16, name="side_tmp")
            t_pre = sm_p.tile([H, 2, 2], F16, name="t_pre")
            nc.sync.dma_start(t_pre[:],
                              rs_out[:, :, 0, S:S + 2]
                              .rearrange("s h x -> h s x"))
            hsum = sm_p.tile([H, 2, R, SP2], F16, name="hsum")
            nc.sync.dma_start(hsum[:], rs_out.rearrange("s h r x -> h s r x"))
            t_sums = sm_p.tile([SH, 2, 2], F32, name="t_sums")
            nc.vector.tensor_copy(t_sums[:], t_pre[:])
            for sd in range(2):
                for r in range(R):
                    row = RH + r * H
                    blk, off = divmod(row, 128)
                    if r % 2 == 0:
                        nc.scalar.activation(catT[sd][off:off + H, blk, :],
                                             hsum[:, sd, r, 0:S],
                                             AF.Relu, scale=HID_SC)
                    else:
                        nc.vector.tensor_scalar(
                            catT[sd][off:off + H, blk, :], hsum[:, sd, r, 0:S],
                            HID_SC, 0.0, op0=ALU.mult, op1=ALU.max)
            sc2, sh2 = bn_from_sums("sB", t_sums[:, :, 0], t_sums[:, :, 1],
                                    gbs_sb[:, 0:2], gbs_sb[:, 2:4], U, SH)
            for sd in range(2):
                nc.scalar.activation(side_tmp[:, sd, :], s_loc[:, sd, :],
                                     AF.Relu, bias=sh2[:, sd:sd + 1],
                                     scale=sc2[:, sd:sd + 1])
                nc.vector.tensor_mul(catT[sd][0:SH, 5, :], side_tmp[:, sd, :],
                                     mask_sb[:])

            # ============ cat matmul (f16) + slim AG ============
            y_sb = [sm_p.tile([O, S], F32, name=f"y_sb{sd}") for sd in range(2)]
            junk2 = sm_p.tile([O, 2, S], F32, name="junk2")
            stats = sm_p.tile([O, 4], F32, name="stats")

            for sd in range(2):
                ps_y = psmm.tile([O, S], F32, tag="mm", name="ps_y")
                for b in range(6):
                    nc.tensor.matmul(ps_y[:], wcat_sb[:, b, sd, :],
                                     catT[sd][:, b, :],
                                     start=(b == 0), stop=(b == 5))
                nc.vector.tensor_copy(y_sb[sd][:], ps_y[:])
                nc.vector.reduce_sum(stats[:, sd:sd + 1], y_sb[sd][:],
                                     axis=AXX)
                nc.gpsimd.tensor_mul(junk2[0:O, sd, :], y_sb[sd][:],
                                     y_sb[sd][:])
                nc.vector.reduce_sum(stats[:, 2 + sd:3 + sd],
                                     junk2[0:O, sd, :], axis=AXX)
            ag_st = sm_p.tile([O, S + 4], F32, name="ag_st")
            nc.vector.tensor_copy(ag_st[:, 0:S], y_sb[1][:])
            nc.vector.tensor_copy(ag_st[:, S:S + 4], stats[:])
            nc.sync.dma_start(ag_in[:], ag_st[:])
            nc.gpsimd.collective_compute("AllGather", ALU.bypass,
                                         replica_groups=replica,
                                         ins=[ag_in.opt()],
                                         outs=[ag_out.opt()])

            yv_all = sm_p.tile([O, NC, S + 4], F32, name="yv_all")
            nc.sync.dma_start(yv_all[:, :, S:S + 4],
                              ag_out[:, :, S:S + 4].rearrange("c p x -> p c x"))
            nc.sync.dma_start(yv_all[:, 0:4, 0:S],
                              ag_out[0:4, :, 0:S].rearrange("c p x -> p c x"))
            nc.sync.dma_start(yv_all[:, 4:8, 0:S],
                              ag_out[4:8, :, 0:S].rearrange("c p x -> p c x"))

            # ============ cat BN (global stats) + embeds ============
            statacc = sm_p.tile([O, 4], F32, name="statacc")
            stat_b = sm_p.tile([O, 4], F32, name="stat_b")
            nc.vector.tensor_add(statacc[:], yv_all[:, 0, S:S + 4],
                                 yv_all[:, 1, S:S + 4])
            nc.gpsimd.tensor_add(stat_b[:], yv_all[:, 4, S:S + 4],
                                 yv_all[:, 5, S:S + 4])
            for c in (2, 3):
                nc.vector.tensor_add(statacc[:], statacc[:],
                                     yv_all[:, c, S:S + 4])
            for c in (6, 7):
                nc.gpsimd.tensor_add(stat_b[:], stat_b[:],
                                     yv_all[:, c, S:S + 4])
            nc.vector.tensor_add(statacc[:], statacc[:], stat_b[:])
            embed_u = sm_p.tile([O, S], F16)
            embed_v = sm_p.tile([O, UP], F16)
            scc, shc = bn_from_sums("cB", statacc[:, 0:2], statacc[:, 2:4],
                                    gbc_sb[:, 0:2], gbc_sb[:, 2:4], U, O)
            nc.scalar.activation(embed_u[:], y_sb[0][:], AF.Relu,
                                 bias=shc[:, 0:1], scale=scc[:, 0:1])
            ev = embed_v.rearrange("p (c u) -> p c u", c=NC)
            nc.scalar.activation(ev[:, 0:4, :], yv_all[:, 0:4, 0:S],
                                 AF.Relu, bias=shc[:, 1:2],
                                 scale=scc[:, 1:2])
            ev_t = sm_p.tile([O, 4, S], F32, name="ev_t")
            nc.vector.tensor_scalar(ev_t[:], yv_all[:, 4:8, 0:S],
                                    scc[:, 1:2], shc[:, 1:2],
                                    op0=ALU.mult, op1=ALU.add)
            nc.vector.tensor_scalar(ev[:, 4:8, :], ev_t[:], 0.0, None,
                                    op0=ALU.max)

            # ============ score ============
            for r in range(R):
                ps_t1 = psmm.tile([O, S], F32, tag="mm", name="ps_t1")
                nc.tensor.matmul(ps_t1[:], q_sb[:, r, :], embed_u[:],
                                 start=True, stop=True)
                t1 = sc_p.tile([O, S], F16, tag="t1", name="t1")
                nc.vector.tensor_copy(t1[:], ps_t1[:])
                for ch in range(2):
                    out_sb = sc_p.tile([128, V], F16, tag="osb", name="out_sb")
                    for i, (n0, nn) in enumerate(NTILES):
                        ps_sc = pssc.tile([128, 512], F32, tag="sc",
                                          name="ps_sc")
                        nc.tensor.matmul(ps_sc[:, 0:nn],
                                         t1[:, ch * 128:(ch + 1) * 128],
                                         embed_v[:, n0:n0 + nn],
                                         start=True, stop=True)
                        if i % 2 == 0:
                            nc.vector.tensor_copy(out_sb[:, n0:n0 + nn],
                                                  ps_sc[:, 0:nn])
                        else:
                            nc.scalar.copy(out_sb[:, n0:n0 + nn],
                                           ps_sc[:, 0:nn])
                    nc.sync.dma_start(score_d[r, ch * 128:(ch + 1) * 128, :],
                                      out_sb[:])

    nc.compile()
    return nc


def _prep(inputs):
    """Host-side shard/pad/scale/cast/pack. Returns in_maps for 8 cores."""
    def padto(a, n, axis):
        pad = [(0, 0)] * a.ndim
        pad[axis] = (0, n - a.shape[axis])
        return np.pad(a, pad)

    f32 = np.float32
    fu = padto(padto(np.asarray(inputs['feature_u'], f32), UP, 0), UP, 1)
    fv = padto(padto(np.asarray(inputs['feature_v'], f32), UP, 0), UP, 1)
    Mu = padto(padto(np.asarray(inputs['M_u'], f32), UP, 1), UP, 2) * M_SC
    Mv = padto(padto(np.asarray(inputs['M_v'], f32), UP, 1), UP, 2) * M_SC
    W = padto(np.asarray(inputs['W'], f32), UP, 1)
    sfu = padto(np.asarray(inputs['side_feature_u'], f32), UP, 0)
    sfv = padto(np.asarray(inputs['side_feature_v'], f32), UP, 0)

    # catT row order [pre | hidden | side]; reference cat order is
    # [hidden | f@W | side] -> permute w_cat rows to match.
    perm = np.concatenate([np.arange(RH, 2 * RH), np.arange(0, RH),
                           np.arange(2 * RH, 2 * RH + SH)])
    wcat = np.stack(
        [padto(np.asarray(inputs[f'w_cat_{s}'], f32)[perm], 6 * 128, 0)
         for s in ('u', 'v')], 1)                   # [768, 2, 75]
    wcat16 = np.ascontiguousarray(
        wcat.reshape(6, 128, 2, O).transpose(1, 0, 2, 3)).astype(F16N)
    wside = np.stack([np.asarray(inputs['w_side_u'], f32),
                      np.asarray(inputs['w_side_v'], f32)], 1).astype(F16N)
    gbs = np.stack([inputs['g_side_u'], inputs['g_side_v'],
                    inputs['beta_side_u'], inputs['beta_side_v']],
                   1).astype(f32)
    gbc = np.stack([inputs['g_cat_u'], inputs['g_cat_v'],
                    inputs['beta_cat_u'], inputs['beta_cat_v']],
                   1).astype(f32)
    # W repacked to [p, k, r*h] so each k-slice is a contiguous [128, RH] rhs
    w16 = np.ascontiguousarray(
        W.reshape(R, KT, 128, H).transpose(2, 1, 0, 3)).reshape(
        128, KT, RH).astype(F16N)
    q16 = np.ascontiguousarray(
        np.asarray(inputs['Q'], f32).transpose(1, 0, 2)).astype(F16N)

    def pack_f(feat, sl):
        # [2048, 256] rows sl -> [128, 16, 256]: f = k*128 + p
        a = np.ascontiguousarray(feat[sl].T)        # [2048 f, 256]
        return np.ascontiguousarray(
            a.reshape(KT, 128, S).transpose(1, 0, 2)).astype(F16N)

    def pack_m(Msc, r, sl):
        # M[r][:, my rows].T -> [128, 2, 2048]: local row = t*128 + p
        a = np.ascontiguousarray(Msc[r][:, sl].T)   # [256 local, 2048]
        return np.ascontiguousarray(
            a.reshape(2, 128, UP).transpose(1, 0, 2)).astype(F8N)

    in_maps = []
    for c in range(NC):
        sl = slice(c * S, (c + 1) * S)
        in_maps.append({
            "fuT": pack_f(fu, sl),
            "fvT": pack_f(fv, sl),
            "muT": np.stack([pack_m(Mu, r, sl) for r in range(R)]),
            "mvT": np.stack([pack_m(Mv, r, sl) for r in range(R)]),
            "w": w16,
            "q": q16,
            "sfT": np.ascontiguousarray(
                np.stack([sfu[sl].T, sfv[sl].T], 1)).astype(F16N),
            "wside": wside,
            "wcat": wcat16,
            "gb_side": gbs,
            "gb_cat": gbc,
            "ident": np.eye(128, dtype=F16N),
            "mask": np.broadcast_to(
                (np.arange(c * S, (c + 1) * S) < U).astype(F16N),
                (SH, S)).copy(),
        })
    return in_maps


def kernel(**inputs) -> np.ndarray:
    if "nc" not in _CACHE:
        _CACHE["nc"] = _build()
    nc = _CACHE["nc"]
    in_maps = _prep(inputs)
    res = bass_utils.run_bass_kernel_spmd(nc, in_maps, core_ids=list(range(NC)))
    score = np.concatenate(
        [np.asarray(res.results[c]["score"]) for c in range(NC)],
        axis=1).astype(np.float32)
    return score[:, :U, :]


if __name__ == "__main__":
    print("kernel module OK")


# revision 54
# speedup vs baseline: 1.0635x; 1.0032x over previous
"""Trainium2 Bass kernel for nn_GCMC (GNN message passing / GCMC scoring).

v5 strategy: row-shard users AND items across 8 NeuronCores (256 padded rows
each). Message passing is ONE merged ReduceScatter: each core column-shards
M (its 256 v-columns of M_u, u-columns of M_v), computes partial hidden sums
for ALL opposite-side rows from its local projection slice, and a single
ReduceScatter (add) over a [NC, 2, H, R, 258] fp16 payload returns both
sides' per-core hidden rows (side-branch BatchNorm partial sums ride in 2
extra columns). One slim AllGather then shares the pre-BN v-side cat output
y_v plus both sides' cat BatchNorm partial sums, so every core computes
global BatchNorm stats locally and the full embed_v for the final bilinear
score.

Precision: fp16 operands (the PE accumulates in fp32; fp16 measured no worse
than bf16 here), fp32 catT is not needed (fp16 catT/w_cat measured fine), M
and the projection copy it contracts with travel as fp8e4m3 (x64 / x4
scales, undone by the hidden-relu activation scale 2^-8) enabling DoubleRow
matmuls and halving the dominant M DMA traffic. The RS payload is fp16; the
AllGather payload MUST stay fp32 - 16-bit AllGather payloads measurably
degrade (~1.5% extra score error, consistent with the collective
round-tripping 16-bit data through bf16), while the fp16 ReduceScatter only
costs ~0.35%. Measured end-to-end max-rel error: 1.41% vs the 2% gate.

Collectives: 1x fp16 ReduceScatter (23.3us) + 1x fp32 AllGather (30.6us) on
the serial collective device, vs 2 RS + 1 AG = 77us in v2.1. A dummy Sqrt
activation at t=0 preloads the activation-function table so the BatchNorm
Sqrt does not pay a 1.3us table load on the post-RS critical path.
"""
import sys
if '/opt/trn_rl_repo' not in sys.path:
    sys.path.insert(0, '/opt/trn_rl_repo')

import numpy as np
import ml_dtypes

import concourse.bass as bass
import concourse.bacc as bacc
import concourse.mybir as mybir
import concourse.tile as tile
from concourse import bass_utils

F16N = np.float16
F8N = ml_dtypes.float8_e4m3
F32 = mybir.dt.float32
F16 = mybir.dt.float16
FP8 = mybir.dt.float8e4
AF = mybir.ActivationFunctionType
ALU = mybir.AluOpType
AXX = mybir.AxisListType.X
DR = mybir.MatmulPerfMode.DoubleRow

U = V = F = 2000
R, H, O, SH, SF = 5, 64, 75, 64, 128
RH = R * H           # 320
UP = 2048            # padded U/V/F
S = 256              # rows per core
SP2 = S + 2          # RS payload row width (256 data + 2 BN-sum cols)
NC = 8
KT = 16              # 128-row k-tiles over the padded 2048 contraction dims
EPS = 1e-5
M_SC, P_SC = 64.0, 4.0          # fp8 scales for M and prevh
HID_SC = 1.0 / (M_SC * P_SC)    # 2^-8, folded into hidden relu
NTILES = [(0, 512), (512, 512), (1024, 512), (1536, 464)]  # score v-tiles

_CACHE = {}


def _build():
    nc = bacc.Bacc("TRN2", target_bir_lowering=False, debug=False,
                   num_devices=NC)

    def din(name, shape, dt):
        return nc.dram_tensor(name, list(shape), dt, kind="ExternalInput").ap()

    fuT_d = din("fuT", (128, KT, S), F16)     # my u rows, [f, kt, u]
    fvT_d = din("fvT", (128, KT, S), F16)
    muT_d = din("muT", (R, 128, 2, UP), FP8)  # x64 M_u[r][:, my_v].T packed
    mvT_d = din("mvT", (R, 128, 2, UP), FP8)
    w_d = din("w", (128, KT, RH), F16)
    q_d = din("q", (O, R, O), F16)
    sfT_d = din("sfT", (SF, 2, S), F16)
    wside_d = din("wside", (SF, 2, SH), F16)
    wcat_d = din("wcat", (128, 6, 2, O), F16)  # rows: pre|hidden|side
    gbs_d = din("gb_side", (SH, 4), F32)
    gbc_d = din("gb_cat", (O, 4), F32)
    ident_d = din("ident", (128, 128), F16)
    mask_d = din("mask", (SH, S), F16)

    score_d = nc.dram_tensor("score", [R, S, V], F16,
                             kind="ExternalOutput").ap()

    with tile.TileContext(nc) as tc:
        with tc.tile_pool(name="const", bufs=1) as const_p, \
             tc.tile_pool(name="big", bufs=1) as big_p, \
             tc.tile_pool(name="mring", bufs=10) as m_p, \
             tc.tile_pool(name="small", bufs=1) as sm_p, \
             tc.tile_pool(name="scoresb", bufs=3) as sc_p, \
             tc.tile_pool(name="psmm", bufs=4, space="PSUM") as psmm, \
             tc.tile_pool(name="pssc", bufs=3, space="PSUM") as pssc, \
             tc.tile_pool(name="dram", bufs=1, space="DRAM") as dram_p:

            # ============ constant/small loads (SP queue) ============
            ident = const_p.tile([128, 128], F16)
            nc.sync.dma_start(ident[:], ident_d)
            eps_t = const_p.tile([128, 1], F32)
            nc.vector.memset(eps_t[:], EPS)
            sqrt_warm = const_p.tile([128, 1], F32, name="sqrt_warm")
            nc.scalar.activation(sqrt_warm[:], eps_t[:], AF.Sqrt,
                                 bias=eps_t[:])
            sfT_sb = const_p.tile([SF, 2, S], F16)
            nc.sync.dma_start(sfT_sb[:], sfT_d)
            wside_sb = const_p.tile([SF, 2, SH], F16)
            nc.sync.dma_start(wside_sb[:], wside_d)
            q_sb = const_p.tile([O, R, O], F16)
            nc.sync.dma_start(q_sb[:], q_d)
            wcat_sb = const_p.tile([128, 6, 2, O], F16)
            nc.sync.dma_start(wcat_sb[:], wcat_d)
            gbs_sb = const_p.tile([SH, 4], F32)
            nc.sync.dma_start(gbs_sb[:], gbs_d)
            gbc_sb = const_p.tile([O, 4], F32)
            nc.sync.dma_start(gbc_sb[:], gbc_d)
            mask_sb = const_p.tile([SH, S], F16)
            nc.sync.dma_start(mask_sb[:], mask_d)

            # ============ big stream (ACT queue, exact order) ============
            w_sb = big_p.tile([128, KT, RH], F16)
            nc.scalar.dma_start(w_sb[:], w_d)
            fvT_sb = big_p.tile([128, KT, S], F16)
            nc.scalar.dma_start(fvT_sb[:, :, 0:128], fvT_d[:, :, 0:128])
            nc.scalar.dma_start(fvT_sb[:, :, 128:S], fvT_d[:, :, 128:S])
            fuT_sb = big_p.tile([128, KT, S], F16)
            nc.scalar.dma_start(fuT_sb[:], fuT_d)
            muT_sb = [m_p.tile([128, 2, UP], FP8, tag="mT", name=f"muT_{r}")
                      for r in range(R)]
            mvT_sb = [m_p.tile([128, 2, UP], FP8, tag="mT", name=f"mvT_{r}")
                      for r in range(R)]
            for r in range(R):
                nc.scalar.dma_start(muT_sb[r][:], muT_d[r])
            for r in range(R):
                nc.scalar.dma_start(mvT_sb[r][:], mvT_d[r])

            # ============ collective buffers ============
            replica = [list(range(NC))]
            rs_in = dram_p.tile([NC, 2, H, R, SP2], F16)
            rs_out = dram_p.tile([2, H, R, SP2], F16)
            ag_in = dram_p.tile([O, S + 4], F32)
            ag_out = dram_p.tile([NC, O, S + 4], F32, addr_space="Shared")

            # catT: [128, 6, S] f16 per side; rows pre(0:320)|hidden|side
            catT = [big_p.tile([128, 6, S], F16, name=f"catT{sd}")
                    for sd in range(2)]
            for sd in range(2):
                nc.vector.memset(catT[sd][SH:128, 5, :], 0.0)

            # partial-hidden staging, one tile per side (avoids WAR
            # serialization of v-side copies behind u-side stage DMAs)
            stages = [big_p.tile([H, NC, R, SP2], F16, name=f"stage{sd}")
                      for sd in range(2)]
            for sd in range(2):
                nc.vector.memset(stages[sd][:, :, :, S:SP2], 0.0)

            # ============ side matmuls + BN partial sums ============
            s_loc = sm_p.tile([SH, 2, S], F32)
            junk = sm_p.tile([128, S], F32, name="junk")

            s_sums = [sm_p.tile([SH, 2], F32, name=f"s_sums{sd}")
                      for sd in range(2)]

            def side_compute(sd):
                ps_s = psmm.tile([SH, S], F32, tag="mm", name="ps_side")
                nc.tensor.matmul(ps_s[:], wside_sb[:, sd, :], sfT_sb[:, sd, :],
                                 start=True, stop=True)
                nc.vector.tensor_copy(s_loc[:, sd, :], ps_s[:])
                nc.vector.reduce_sum(s_sums[sd][:, 0:1], s_loc[:, sd, :],
                                     axis=AXX)
                nc.vector.tensor_mul(junk[0:SH, :], s_loc[:, sd, :],
                                     s_loc[:, sd, :])
                nc.vector.reduce_sum(s_sums[sd][:, 1:2], junk[0:SH, :],
                                     axis=AXX)

            def side_stage(sd):
                # replicate into every dest shard of the RS payload (row r=0)
                for c in range(NC):
                    nc.vector.tensor_copy(stages[sd][:, c, 0, S:S + 2],
                                          s_sums[sd][:])

            # ============ projection: pre[row, rh] = f^T W ============
            prevh = [big_p.tile([128, 2, RH], F16, name=f"prevh_{sd}")
                     for sd in range(2)]
            prevh8 = [big_p.tile([128, 2, RH], FP8, name=f"prevh8_{sd}")
                      for sd in range(2)]

            def proj_side(sd, fT):
                pre_f16 = prevh[sd]
                for ch in range(2):
                    ps_pre = psmm.tile([128, RH], F32, tag="mm", name="ps_pre")
                    for k in range(KT):
                        nc.tensor.matmul(ps_pre[:],
                                         fT[:, k, ch * 128:(ch + 1) * 128],
                                         w_sb[:, k, :],
                                         start=(k == 0), stop=(k == KT - 1))
                    nc.vector.tensor_copy(pre_f16[:, ch, :], ps_pre[:])
                    nc.scalar.activation(prevh8[sd][:, ch, :], ps_pre[:],
                                         AF.Copy, scale=P_SC)

            # deferred: transpose prevh into catT pre rows during the RS
            # window (keeps these copies off the pre-RS DVE critical path)
            def catT_pre_fill(sd):
                for ch in range(2):
                    for c in range(3):
                        cw = min(128, RH - c * 128)
                        ps_t = psmm.tile([128, 128], F16, tag="mm",
                                         name="ps_t")
                        nc.tensor.transpose(
                            ps_t[0:cw, :],
                            prevh[sd][:, ch, c * 128:c * 128 + cw],
                            ident[:])
                        row = c * 128
                        blk, off = divmod(row, 128)
                        eng = nc.vector if (ch + c) % 2 == 0 else None
                        if eng is None:
                            nc.scalar.copy(
                                catT[sd][off:off + cw, blk,
                                         ch * 128:(ch + 1) * 128],
                                ps_t[0:cw, :])
                        else:
                            nc.vector.tensor_copy(
                                catT[sd][off:off + cw, blk,
                                         ch * 128:(ch + 1) * 128],
                                ps_t[0:cw, :])

            # ==== partial hidden (fp8 DoubleRow) -> f16 stage -> DMA ====
            def partial_side(sd, mT, osd):
                stage = stages[sd]
                for r in range(R):
                    pss = [psmm.tile([H, 2, S], F32, tag="mm", name=f"ps_p{g}")
                           for g in range(4)]
                    for g in range(4):
                        for c in range(2):
                            nc.tensor.matmul(
                                pss[g][:, c, :],
                                prevh8[osd][:, :, r * H:(r + 1) * H],
                                mT[r][:, :, (2 * g + c) * S:(2 * g + c + 1) * S],
                                start=True, stop=True, perf_mode=DR)
                    for g in range(4):
                        dst = stage[:, 2 * g:2 * g + 2, r, 0:S]
                        if g % 2 == 0:
                            nc.vector.tensor_copy(dst, pss[g][:])
                        else:
                            nc.scalar.copy(dst, pss[g][:])
                    nc.sync.dma_start(
                        rs_in[:, sd, :, r, :].rearrange("c h x -> h c x"),
                        stage[:, :, r, :])

            side_compute(0)
            side_compute(1)
            proj_side(1, fvT_sb)
            proj_side(0, fuT_sb)
            side_stage(0)
            partial_side(0, muT_sb, 1)
            side_stage(1)
            partial_side(1, mvT_sb, 0)
            catT_pre_fill(1)
            catT_pre_fill(0)
            nc.gpsimd.collective_compute("ReduceScatter", ALU.add,
                                         replica_groups=replica,
                                         ins=[rs_in.opt()],
                                         outs=[rs_out.opt()])



            # ============ BN helpers (both sides batched: [P, 2]) ======
            def bn_from_sums(tg, sums, sumsq, g_col, b_col, n, P):
                def t(nm):
                    return sm_p.tile([P, 2], F32, tag=f"{nm}_{tg}",
                                     name=f"{nm}_{tg}")
                mu = t("bn_mu")
                nc.vector.tensor_scalar_mul(mu[:], sums[:], 1.0 / n)
                e2 = t("bn_e2")
                nc.vector.tensor_scalar_mul(e2[:], sumsq[:], 1.0 / n)
                var = t("bn_var")
                nc.vector.tensor_mul(var[:], mu[:], mu[:])
                nc.vector.tensor_sub(var[:], e2[:], var[:])
                std = t("bn_std")
                nc.scalar.activation(std[:], var[:], AF.Sqrt, bias=eps_t[0:P, :])
                rstd = t("bn_rstd")
                nc.vector.reciprocal(rstd[:], std[:])
                scale = t("bn_scale")
                nc.vector.tensor_mul(scale[:], g_col, rstd[:])
                shift = t("bn_shift")
                nc.vector.tensor_mul(shift[:], mu[:], scale[:])
                nc.vector.tensor_sub(shift[:], b_col, shift[:])
                return scale, shift

            # ====== hidden relu into catT rows 320:640; side BN ======
            side_tmp = sm_p.tile([SH, 2, S], F16, name="side_tmp")
            t_pre = sm_p.tile([H, 2, 2], F16, name="t_pre")
            nc.sync.dma_start(t_pre[:],
                              rs_out[:, :, 0, S:S + 2]
                              .rearrange("s h x -> h s x"))
            hsum = sm_p.tile([H, 2, R, SP2], F16, name="hsum")
            nc.sync.dma_start(hsum[:], rs_out.rearrange("s h r x -> h s r x"))
            t_sums = sm_p.tile([SH, 2, 2], F32, name="t_sums")
            nc.vector.tensor_copy(t_sums[:], t_pre[:])
            for sd in range(2):
                for r in range(R):
                    row = RH + r * H
                    blk, off = divmod(row, 128)
                    if r % 2 == 0:
                        nc.scalar.activation(catT[sd][off:off + H, blk, :],
                                             hsum[:, sd, r, 0:S],
                                             AF.Relu, scale=HID_SC)
                    else:
                        nc.vector.tensor_scalar(
                            catT[sd][off:off + H, blk, :], hsum[:, sd, r, 0:S],
                            HID_SC, 0.0, op0=ALU.mult, op1=ALU.max)
            sc2, sh2 = bn_from_sums("sB", t_sums[:, :, 0], t_sums[:, :, 1],
                                    gbs_sb[:, 0:2], gbs_sb[:, 2:4], U, SH)
            nc.scalar.activation(side_tmp[:, 0, :], s_loc[:, 0, :],
                                 AF.Relu, bias=sh2[:, 0:1], scale=sc2[:, 0:1])
            sv_t = sm_p.tile([SH, S], F32, name="sv_t")
            nc.vector.tensor_scalar(sv_t[:], s_loc[:, 1, :], sc2[:, 1:2],
                                    sh2[:, 1:2], op0=ALU.mult, op1=ALU.add)
            nc.vector.tensor_scalar(side_tmp[:, 1, :], sv_t[:], 0.0, None,
                                    op0=ALU.max)
            for sd in range(2):
                nc.vector.tensor_mul(catT[sd][0:SH, 5, :], side_tmp[:, sd, :],
                                     mask_sb[:])

            # ============ cat matmul (f16) + slim AG ============
            y_sb = [sm_p.tile([O, S], F32, name=f"y_sb{sd}") for sd in range(2)]
            junk2 = sm_p.tile([O, 2, S], F32, name="junk2")
            stats = sm_p.tile([O, 4], F32, name="stats")

            for sd in range(2):
                ps_y = psmm.tile([O, S], F32, tag="mm", name="ps_y")
                for b in range(6):
                    nc.tensor.matmul(ps_y[:], wcat_sb[:, b, sd, :],
                                     catT[sd][:, b, :],
                                     start=(b == 0), stop=(b == 5))
                nc.vector.tensor_copy(y_sb[sd][:], ps_y[:])
                nc.vector.reduce_sum(stats[:, sd:sd + 1], y_sb[sd][:],
                                     axis=AXX)
                nc.gpsimd.tensor_mul(junk2[0:O, sd, :], y_sb[sd][:],
                                     y_sb[sd][:])
                nc.vector.reduce_sum(stats[:, 2 + sd:3 + sd],
                                     junk2[0:O, sd, :], axis=AXX)
            ag_st = sm_p.tile([O, S + 4], F32, name="ag_st")
            nc.gpsimd.tensor_copy(ag_st[:, 0:S], y_sb[1][:])
            nc.vector.tensor_copy(ag_st[:, S:S + 4], stats[:])
            nc.sync.dma_start(ag_in[:], ag_st[:])
            nc.gpsimd.collective_compute("AllGather", ALU.bypass,
                                         replica_groups=replica,
                                         ins=[ag_in.opt()],
                                         outs=[ag_out.opt()])

            yv_all = sm_p.tile([O, NC, S + 4], F32, name="yv_all")
            nc.sync.dma_start(yv_all[:, :, S:S + 4],
                              ag_out[:, :, S:S + 4].rearrange("c p x -> p c x"))
            nc.sync.dma_start(yv_all[:, 0:4, 0:S],
                              ag_out[0:4, :, 0:S].rearrange("c p x -> p c x"))
            nc.sync.dma_start(yv_all[:, 4:8, 0:S],
                              ag_out[4:8, :, 0:S].rearrange("c p x -> p c x"))

            # ============ cat BN (global stats) + embeds ============
            statacc = sm_p.tile([O, 4], F32, name="statacc")
            stat_b = sm_p.tile([O, 4], F32, name="stat_b")
            nc.vector.tensor_add(statacc[:], yv_all[:, 0, S:S + 4],
                                 yv_all[:, 1, S:S + 4])
            nc.gpsimd.tensor_add(stat_b[:], yv_all[:, 4, S:S + 4],
                                 yv_all[:, 5, S:S + 4])
            for c in (2, 3):
                nc.vector.tensor_add(statacc[:], statacc[:],
                                     yv_all[:, c, S:S + 4])
            for c in (6, 7):
                nc.gpsimd.tensor_add(stat_b[:], stat_b[:],
                                     yv_all[:, c, S:S + 4])
            nc.vector.tensor_add(statacc[:], statacc[:], stat_b[:])
            embed_u = sm_p.tile([O, S], F16)
            embed_v = sm_p.tile([O, UP], F16)
            scc, shc = bn_from_sums("cB", statacc[:, 0:2], statacc[:, 2:4],
                                    gbc_sb[:, 0:2], gbc_sb[:, 2:4], U, O)
            nc.scalar.activation(embed_u[:], y_sb[0][:], AF.Relu,
                                 bias=shc[:, 0:1], scale=scc[:, 0:1])
            ev = embed_v.rearrange("p (c u) -> p c u", c=NC)
            nc.scalar.activation(ev[:, 0:4, :], yv_all[:, 0:4, 0:S],
                                 AF.Relu, bias=shc[:, 1:2],
                                 scale=scc[:, 1:2])
            ev_t = sm_p.tile([O, 4, S], F32, name="ev_t")
            nc.vector.tensor_scalar(ev_t[:], yv_all[:, 4:8, 0:S],
                                    scc[:, 1:2], shc[:, 1:2],
                                    op0=ALU.mult, op1=ALU.add)
            nc.vector.tensor_scalar(ev[:, 4:8, :], ev_t[:], 0.0, None,
                                    op0=ALU.max)

            # ============ score ============
            for r in range(R):
                ps_t1 = psmm.tile([O, S], F32, tag="mm", name="ps_t1")
                nc.tensor.matmul(ps_t1[:], q_sb[:, r, :], embed_u[:],
                                 start=True, stop=True)
                t1 = sc_p.tile([O, S], F16, tag="t1", name="t1")
                nc.vector.tensor_copy(t1[:], ps_t1[:])
                for ch in range(2):
                    out_sb = sc_p.tile([128, V], F16, tag="osb", name="out_sb")
                    for i, (n0, nn) in enumerate(NTILES):
                        ps_sc = pssc.tile([128, 512], F32, tag="sc",
                                          name="ps_sc")
                        nc.tensor.matmul(ps_sc[:, 0:nn],
                                         t1[:, ch * 128:(ch + 1) * 128],
                                         embed_v[:, n0:n0 + nn],
                                         start=True, stop=True)
                        if i % 2 == 0:
                            nc.vector.tensor_copy(out_sb[:, n0:n0 + nn],
                                                  ps_sc[:, 0:nn])
                        else:
                            nc.scalar.copy(out_sb[:, n0:n0 + nn],
                                           ps_sc[:, 0:nn])
                    nc.sync.dma_start(score_d[r, ch * 128:(ch + 1) * 128, :],
                                      out_sb[:])

    nc.compile()
    return nc


def _prep(inputs):
    """Host-side shard/pad/scale/cast/pack. Returns in_maps for 8 cores."""
    def padto(a, n, axis):
        pad = [(0, 0)] * a.ndim
        pad[axis] = (0, n - a.shape[axis])
        return np.pad(a, pad)

    f32 = np.float32
    fu = padto(padto(np.asarray(inputs['feature_u'], f32), UP, 0), UP, 1)
    fv = padto(padto(np.asarray(inputs['feature_v'], f32), UP, 0), UP, 1)
    Mu = padto(padto(np.asarray(inputs['M_u'], f32), UP, 1), UP, 2) * M_SC
    Mv = padto(padto(np.asarray(inputs['M_v'], f32), UP, 1), UP, 2) * M_SC
    W = padto(np.asarray(inputs['W'], f32), UP, 1)
    sfu = padto(np.asarray(inputs['side_feature_u'], f32), UP, 0)
    sfv = padto(np.asarray(inputs['side_feature_v'], f32), UP, 0)

    # catT row order [pre | hidden | side]; reference cat order is
    # [hidden | f@W | side] -> permute w_cat rows to match.
    perm = np.concatenate([np.arange(RH, 2 * RH), np.arange(0, RH),
                           np.arange(2 * RH, 2 * RH + SH)])
    wcat = np.stack(
        [padto(np.asarray(inputs[f'w_cat_{s}'], f32)[perm], 6 * 128, 0)
         for s in ('u', 'v')], 1)                   # [768, 2, 75]
    wcat16 = np.ascontiguousarray(
        wcat.reshape(6, 128, 2, O).transpose(1, 0, 2, 3)).astype(F16N)
    wside = np.stack([np.asarray(inputs['w_side_u'], f32),
                      np.asarray(inputs['w_side_v'], f32)], 1).astype(F16N)
    gbs = np.stack([inputs['g_side_u'], inputs['g_side_v'],
                    inputs['beta_side_u'], inputs['beta_side_v']],
                   1).astype(f32)
    gbc = np.stack([inputs['g_cat_u'], inputs['g_cat_v'],
                    inputs['beta_cat_u'], inputs['beta_cat_v']],
                   1).astype(f32)
    # W repacked to [p, k, r*h] so each k-slice is a contiguous [128, RH] rhs
    w16 = np.ascontiguousarray(
        W.reshape(R, KT, 128, H).transpose(2, 1, 0, 3)).reshape(
        128, KT, RH).astype(F16N)
    q16 = np.ascontiguousarray(
        np.asarray(inputs['Q'], f32).transpose(1, 0, 2)).astype(F16N)

    def pack_f(feat, sl):
        # [2048, 256] rows sl -> [128, 16, 256]: f = k*128 + p
        a = np.ascontiguousarray(feat[sl].T)        # [2048 f, 256]
        return np.ascontiguousarray(
            a.reshape(KT, 128, S).transpose(1, 0, 2)).astype(F16N)

    def pack_m(Msc, r, sl):
        # M[r][:, my rows].T -> [128, 2, 2048]: local row = t*128 + p
        a = np.ascontiguousarray(Msc[r][:, sl].T)   # [256 local, 2048]
        return np.ascontiguousarray(
            a.reshape(2, 128, UP).transpose(1, 0, 2)).astype(F8N)

    in_maps = []
    for c in range(NC):
        sl = slice(c * S, (c + 1) * S)
        in_maps.append({
            "fuT": pack_f(fu, sl),
            "fvT": pack_f(fv, sl),
            "muT": np.stack([pack_m(Mu, r, sl) for r in range(R)]),
            "mvT": np.stack([pack_m(Mv, r, sl) for r in range(R)]),
            "w": w16,
            "q": q16,
            "sfT": np.ascontiguousarray(
                np.stack([sfu[sl].T, sfv[sl].T], 1)).astype(F16N),
            "wside": wside,
            "wcat": wcat16,
            "gb_side": gbs,
            "gb_cat": gbc,
            "ident": np.eye(128, dtype=F16N),
            "mask": np.broadcast_to(
                (np.arange(c * S, (c + 1) * S) < U).astype(F16N),
                (SH, S)).copy(),
        })
    return in_maps


def kernel(**inputs) -> np.ndarray:
    if "nc" not in _CACHE:
        _CACHE["nc"] = _build()
    nc = _CACHE["nc"]
    in_maps = _prep(inputs)
    res = bass_utils.run_bass_kernel_spmd(nc, in_maps, core_ids=list(range(NC)))
    score = np.concatenate(
        [np.asarray(res.results[c]["score"]) for c in range(NC)],
        axis=1).astype(np.float32)
    return score[:, :U, :]


if __name__ == "__main__":
    print("kernel module OK")
